# revision 37
# baseline (speedup 1.0000x reference)
"""Canny edge detector on 8 Trainium2 NeuronCores (Bass/Tile).

Sharding: row slabs. Core i owns output rows [118*i, 118*(i+1)) of ALL 8
images. (The reference's flat gather at B=8 cross-wires images inside NMS:
sel_pos(b,h,w) = dirconv_b(gm_{idx(b,h,w)})(h,w), so every output pixel needs
all 8 images' gradient-magnitude maps at its rows -> shard by rows, not by
image.) The leftover band (rows 944..1023) is computed per-image on the
owning core; gm maps are exchanged through DRAM collectives (AllGather for
plain maps, AllToAll for reader-direction-shifted maps - the per-core shift
must live in data routing because SPMD shares one instruction stream). The
B8 block runs FIRST so both collectives hide under the main-slab conv.

Numerics: the output is a thresholded argmax - iid noise of eps relative
flips ~45k*eps pixels, and the gate (rel 2e-2 ~ 840 flips) needs eps <=
1.5e-5. fp16 or float32r matmuls are far too lossy (measured), BUT fp16
products accumulate exactly into fp32 PSUM, so the conv phase uses
fp16 PAIRS: x = x_hi + x_lo (host-split), bl = sum_h W_h @ {x_hi,x_lo};
blt = blt_hi + blt_lo (Act copy + DVE residual); sobel = exact +-1/2
fp16 weights on both halves. Only systematic error left is the fp16
rounding of the gauss weights, minimized by a host-side scale search
(the scale folds into the LOW/HIGH thresholds; masks are scale-free).
gm / orientation stay fp32; NMS indicator algebra runs in fp16/u8.

SBUF: directions 0-4 run the NMS in a row-shifted frame (tile row r =
thin row p + 1) so their shifted compare operand is the RAW gm tile and
only one aligned copy (ce = gm rows 1..121) is ever made - no dn tiles.

All compute-engine APs must start at partition 0 (HW constraint), so row
re-alignment between pipeline stages is done with SBUF->SBUF DMAs.
"""

import os

os.environ.setdefault("BY_DEFAULT_DISABLE_SUBTILE_DEPS", "1")

import numpy as np

H = 1024
W = 1024
B = 8
NC = 8
SLAB = 118                    # main-slab output rows per core
B8_START = SLAB * NC          # 944
B8_ROWS = H - B8_START        # 80
LOW_T, HIGH_T = 2.5, 5.0
T22SQ = float(np.float32(np.tan(np.pi / 8.0)) ** 2)

DELTAS = {0: (0, 1), 1: (1, 1), 2: (1, 0), 3: (1, -1),
          4: (0, -1), 5: (-1, -1), 6: (-1, 0), 7: (-1, 1)}


def _gauss5():
    n = np.arange(5, dtype=np.float32) - 2.0
    return np.exp(-0.5 * n * n).astype(np.float32)


def _wscale():
    """Scale s minimizing fp16 rounding of the 6 distinct 2D-gauss weights
    s*gi*gj. gm scales by s; the LOW/HIGH thresholds absorb it."""
    if "s" in _CACHE:
        return _CACHE["s"]
    g = _gauss5().astype(np.float64)
    prods = np.array([g[i] * g[j] for i in range(3) for j in range(i, 3)])
    best, bs = 1e9, 1.0
    for s in np.linspace(1.0, 2.0, 65536, endpoint=False):
        r = np.abs(np.float64(np.float16(s * prods)) / (s * prods) - 1.0)
        m = r.max()
        if m < best:
            best, bs = m, float(s)
    _CACHE["s"] = bs
    return bs


def _band(n_in, n_out, offset, taps):
    """M[k, m] = taps[k - m - offset] for k-m-offset in range(len(taps))."""
    m_ = np.zeros((n_in, n_out), np.float32)
    for mm in range(n_out):
        for t, w in enumerate(taps):
            k = mm + offset + t
            if 0 <= k < n_in:
                m_[k, mm] = w
    return m_


def _const_mats(core):
    g = _gauss5()
    s = _wscale()
    sg = [float(np.float16(s * g[0] * gv)) for gv in g]  # pre-rounded row h=0
    mats = {}
    # fused 2D blur: bl = sum_h band(f16(s*g_h*g)) @ x_colshift_h
    for name, gh in (("BV", g[0]), ("BVG1", g[1]), ("BVG2", g[2])):
        taps = [float(np.float16(s * gh * gv)) for gv in g]
        mats[name] = _band(128, 124, 0, taps)
        mats[name + "_8"] = _band(88, 86, 0, taps)
    b121 = _band(124, 122, 0, [1.0, 2.0, 1.0])
    b10m1 = _band(124, 122, 0, [1.0, 0.0, -1.0])
    if core == 0:  # img rows -2,-1 must yield gm=0 (zero-pad semantics)
        b121[:, 0:2] = 0.0
        b10m1[:, 0:2] = 0.0
    mats["B121"] = b121
    mats["B121N"] = -b121
    mats["B10M1"] = b10m1
    mats["B10M1X2"] = 2.0 * b10m1
    # thin frame (dirs 5-7): strong row k <-> thin p=k; mp row p; col0 dummy
    bones = _band(120, 119, -1, [1.0, 1.0, 1.0])
    bones[:, 0] = 0.0
    if core == 0:
        bones[:, 1] = 0.0  # border img row 0
    mats["BONES"] = bones
    # shifted frame (dirs 0-4): strong row k <-> thin p=k-1; mp row m <->
    # thin p=m-1; needs strong k in {m-1,m,m+1}
    bones_d = _band(121, 120, -1, [1.0, 1.0, 1.0])
    bones_d[:, 0:2] = 0.0
    if core == 0:
        bones_d[:, 2] = 0.0  # border img row 0 (thin p=1 -> m=2)
    mats["BONES_D"] = bones_d
    # B8 block: x8 row k <-> img 936+k; bl8 row m <-> img 938+m (86 rows)
    b121_8 = _band(86, 84, 1, [1.0, 2.0, 1.0])
    b10m1_8 = _band(86, 84, 1, [1.0, 0.0, -1.0])
    mats["B121_8"] = b121_8
    mats["B121N_8"] = -b121_8
    mats["B10M1_8"] = b10m1_8
    mats["B10M1X2_8"] = 2.0 * b10m1_8
    bones8 = _band(81, 81, -1, [1.0, 1.0, 1.0])
    bones8[:, 0] = 0.0
    bones8[:, 80] = 0.0  # border row 1023
    mats["BONES8"] = bones8
    return {k: np.ascontiguousarray(v, np.float16) for k, v in mats.items()}


MAT_SPECS = {
    # B8 conv runs first: its matrices load first so PE starts early
    "BV_8": [88, 86], "BVG1_8": [88, 86], "BVG2_8": [88, 86],
    "B121_8": [86, 84], "B121N_8": [86, 84],
    "B10M1_8": [86, 84], "B10M1X2_8": [86, 84],
    "BV": [128, 124], "BVG1": [128, 124], "BVG2": [128, 124],
    "B121": [124, 122], "B121N": [124, 122],
    "B10M1": [124, 122], "B10M1X2": [124, 122],
    "BONES": [120, 119], "BONES_D": [121, 120], "BONES8": [81, 81],
}

_CACHE = {}


def _build_program():
    if "nc" in _CACHE:
        return _CACHE["nc"]
    import concourse.bass as bass
    import concourse.mybir as mybir
    from concourse.tile import TileContext

    f32 = mybir.dt.float32
    f16 = mybir.dt.float16
    u8 = mybir.dt.uint8
    Alu = mybir.AluOpType
    s = _wscale()
    LOW_S, HIGH_S = LOW_T * s, HIGH_T * s

    nc = bass.Bass()

    xh = nc.declare_dram_parameter("xh", [B * 3, 128, W], f16, isOutput=False)
    xl = nc.declare_dram_parameter("xl", [B * 3, 128, W], f16, isOutput=False)
    x8h = nc.declare_dram_parameter("x8h", [3, 88, W], f16, isOutput=False)
    x8l = nc.declare_dram_parameter("x8l", [3, 88, W], f16, isOutput=False)
    mat_d = {k: nc.declare_dram_parameter(k, v, f16, isOutput=False)
             for k, v in MAT_SPECS.items()}
    outm = nc.declare_dram_parameter("outm", [B, SLAB, W], f16, isOutput=True)
    out8 = nc.declare_dram_parameter("out8", [B8_ROWS, W], f16, isOutput=True)

    with TileContext(nc) as tc:
        with (
            tc.tile_pool(name="consts", bufs=1) as cpool,
            tc.tile_pool(name="gmp", bufs=1) as gmpool,
            tc.tile_pool(name="msk", bufs=1) as mskpool,
            tc.tile_pool(name="dram", bufs=1, space="DRAM") as dpool,
        ):
            mt = {}
            for name, shp in MAT_SPECS.items():
                t = cpool.tile(shp, f16, tag=name, name=name)
                nc.sync.dma_start(out=t[:], in_=mat_d[name][:])
                mt[name] = t

            gm_tiles = []
            masks = []
            ce = []
            gm8 = gmpool.tile([85, W + 2], f32, tag="gm8self")
            ce8 = gmpool.tile([81, W], f32, tag="ce8self")
            # =========== conv phase =======================================
            with (
                tc.tile_pool(name="xin", bufs=2) as xpool,
                tc.tile_pool(name="bls", bufs=2) as blspool,
                tc.tile_pool(name="sq", bufs=1) as sqpool,
                tc.tile_pool(name="gsm", bufs=2) as gspool,
                tc.tile_pool(name="mskt", bufs=1) as msktpool,
                tc.tile_pool(name="psA", bufs=2, space="PSUM") as psA,
                tc.tile_pool(name="psB", bufs=2, space="PSUM") as psB,
            ):
                def conv_pipeline(c, n_in, n_bl, n_gxy, bv, bvg1, bvg2,
                                  b121, b121n, b10m1, b10m1x2,
                                  xsrc_h, xsrc_l, sqx_st, sqy_st,
                                  gxs_sb, gys_sb):
                    """One (image, channel): fp16-pair blur + sobel."""
                    xth = xpool.tile([128, W + 4], f16, tag="xh", name="xth")
                    xtl = xpool.tile([128, W + 4], f16, tag="xl", name="xtl")
                    for xt, src in ((xth, xsrc_h), (xtl, xsrc_l)):
                        nc.gpsimd.memset(xt[:, 0:2], 0.0)
                        nc.gpsimd.memset(xt[:, W + 2:W + 4], 0.0)
                        nc.sync.dma_start(out=xt[0:n_in, 2:W + 2], in_=src)
                    bl = psA.tile([124, W], f32, tag="bl", name="bl")
                    lhs5 = [bv, bvg1, bvg2, bvg1, bv]
                    for lo in (0, 512):
                        first = True
                        for xt in (xth, xtl):
                            for h_ in range(5):
                                nc.tensor.matmul(
                                    out=bl[0:n_bl, lo:lo + 512],
                                    lhsT=lhs5[h_][0:n_in, 0:n_bl],
                                    rhs=xt[0:n_in, h_ + lo:h_ + lo + 512],
                                    start=first,
                                    stop=(xt is xtl and h_ == 4))
                                first = False
                    # fp16 pair of bl for the sobel rhs
                    blh = blspool.tile([124, W + 2], f16, tag="blh",
                                       name="blh")
                    bll = blspool.tile([124, W + 2], f16, tag="bll",
                                       name="bll", bufs=1)
                    for t in (blh, bll):
                        nc.gpsimd.memset(t[:, 0:1], 0.0)
                        nc.gpsimd.memset(t[:, W + 1:W + 2], 0.0)
                    nc.scalar.copy(out=blh[0:n_bl, 1:W + 1], in_=bl[0:n_bl, :])
                    nc.vector.tensor_tensor(out=bll[0:n_bl, 1:W + 1],
                                            in0=bl[0:n_bl, :],
                                            in1=blh[0:n_bl, 1:W + 1],
                                            op=Alu.subtract)
                    gx = psB.tile([122, W], f32, tag="gxy", name="gx")
                    gy = psB.tile([122, W], f32, tag="gxy", name="gy")
                    for lo in (0, 512):
                        for i, blt in enumerate((blh, bll)):
                            st = (i == 0)
                            sp = (i == 1)
                            nc.tensor.matmul(
                                out=gx[0:n_gxy, lo:lo + 512],
                                lhsT=b121[0:n_bl, 0:n_gxy],
                                rhs=blt[0:n_bl, lo:lo + 512],
                                start=st, stop=False)
                            nc.tensor.matmul(
                                out=gx[0:n_gxy, lo:lo + 512],
                                lhsT=b121n[0:n_bl, 0:n_gxy],
                                rhs=blt[0:n_bl, 2 + lo:2 + lo + 512],
                                start=False, stop=sp)
                            nc.tensor.matmul(
                                out=gy[0:n_gxy, lo:lo + 512],
                                lhsT=b10m1[0:n_bl, 0:n_gxy],
                                rhs=blt[0:n_bl, 2 + lo:2 + lo + 512],
                                start=st, stop=False)
                            nc.tensor.matmul(
                                out=gy[0:n_gxy, lo:lo + 512],
                                lhsT=b10m1x2[0:n_bl, 0:n_gxy],
                                rhs=blt[0:n_bl, 1 + lo:1 + lo + 512],
                                start=False, stop=False)
                            nc.tensor.matmul(
                                out=gy[0:n_gxy, lo:lo + 512],
                                lhsT=b10m1[0:n_bl, 0:n_gxy],
                                rhs=blt[0:n_bl, lo:lo + 512],
                                start=False, stop=sp)
                    nc.scalar.square(out=sqx_st[0:n_gxy, c * W:(c + 1) * W],
                                     in_=gx[0:n_gxy, :])
                    nc.scalar.square(out=sqy_st[0:n_gxy, c * W:(c + 1) * W],
                                     in_=gy[0:n_gxy, :])
                    # gxs/gys accumulation in f32 SBUF (masks need f32)
                    if c == 0:
                        nc.scalar.copy(out=gxs_sb[0:n_gxy, :],
                                       in_=gx[0:n_gxy, :])
                        nc.scalar.copy(out=gys_sb[0:n_gxy, :],
                                       in_=gy[0:n_gxy, :])
                    else:
                        nc.vector.tensor_tensor(out=gxs_sb[0:n_gxy, :],
                                                in0=gxs_sb[0:n_gxy, :],
                                                in1=gx[0:n_gxy, :],
                                                op=Alu.add)
                        nc.vector.tensor_tensor(out=gys_sb[0:n_gxy, :],
                                                in0=gys_sb[0:n_gxy, :],
                                                in1=gy[0:n_gxy, :],
                                                op=Alu.add)

                def finish_image(n_gxy, sqx_st, sqy_st, gm_t):
                    """magnitude: m2 (Pool), sqrt (Act), gm chunk adds."""
                    nc.gpsimd.tensor_tensor(out=sqx_st[0:n_gxy, :],
                                            in0=sqx_st[0:n_gxy, :],
                                            in1=sqy_st[0:n_gxy, :],
                                            op=Alu.add)
                    # reuse sqy's buffer: m2 (its last reader) just finished
                    mag = sqpool.tile([122, 3 * W], f32, tag="sqy",
                                      name="mag")
                    nc.scalar.sqrt(out=mag[0:n_gxy, :], in_=sqx_st[0:n_gxy, :])
                    gmi = gm_t[0:n_gxy, 1:W + 1]
                    nc.vector.tensor_tensor(out=gmi, in0=mag[0:n_gxy, 0:W],
                                            in1=mag[0:n_gxy, W:2 * W],
                                            op=Alu.add)
                    nc.vector.tensor_tensor(out=gmi, in0=gmi,
                                            in1=mag[0:n_gxy, 2 * W:3 * W],
                                            op=Alu.add)

                def make_masks(gxs_sb, gys_sb, n, shift, n_thin, j):
                    """u8 class masks at conv frame [0:n], DMA-shifted down
                    by `shift` rows into persistent thin-frame tiles."""
                    a2 = gspool.tile([122, W], f32, tag="a2", name="a2",
                                     bufs=1)
                    b2 = gspool.tile([122, W], f32, tag="b2", name="b2",
                                     bufs=1)
                    nc.scalar.square(out=a2[0:n, :], in_=gxs_sb[0:n, :])
                    nc.scalar.square(out=b2[0:n, :], in_=gys_sb[0:n, :])
                    sgx = gspool.tile([122, W], u8, tag="sgx", name="sgx",
                                      bufs=1)
                    sgy = gspool.tile([122, W], u8, tag="sgy", name="sgy",
                                      bufs=1)
                    nc.vector.tensor_scalar(out=sgx[0:n, :],
                                            in0=gxs_sb[0:n, :], scalar1=0.0,
                                            scalar2=None, op0=Alu.is_ge)
                    nc.vector.tensor_scalar(out=sgy[0:n, :],
                                            in0=gys_sb[0:n, :], scalar1=0.0,
                                            scalar2=None, op0=Alu.is_ge)
                    tmp = [msktpool.tile([122, W], u8, tag=t, name=t)
                           for t in ("tc0", "tc2", "tsm")]
                    nc.vector.scalar_tensor_tensor(
                        out=tmp[0][0:n, :], in0=a2[0:n, :], scalar=T22SQ,
                        in1=b2[0:n, :], op0=Alu.mult, op1=Alu.is_gt)
                    nc.vector.scalar_tensor_tensor(
                        out=tmp[1][0:n, :], in0=b2[0:n, :], scalar=T22SQ,
                        in1=a2[0:n, :], op0=Alu.mult, op1=Alu.is_gt)
                    # sign agreement == (ab >= 0) wherever c0/c2 don't apply
                    nc.vector.tensor_tensor(out=tmp[2][0:n, :],
                                            in0=sgx[0:n, :], in1=sgy[0:n, :],
                                            op=Alu.is_equal)
                    out = []
                    for t, tag in zip(tmp, ("c0", "c2", "sm")):
                        p = mskpool.tile([n_thin, W], u8, tag=f"{tag}_{j}",
                                         name=f"{tag}_{j}")
                        nc.sync.dma_start(out=p[:],
                                          in_=t[shift:shift + n_thin, :])
                        out.append(p)
                    return out

                # ---- B8 block FIRST so the collectives hide under main conv
                nc.vector.memset(gm8[:], 0.0)
                sqx8 = sqpool.tile([122, 3 * W], f32, tag="sqx", name="sqx")
                sqy8 = sqpool.tile([122, 3 * W], f32, tag="sqy", name="sqy")
                gxs8 = gspool.tile([122, W], f32, tag="gxs", name="gxs")
                gys8 = gspool.tile([122, W], f32, tag="gys", name="gys")
                for c in range(3):
                    conv_pipeline(c, 88, 86, 84, mt["BV_8"], mt["BVG1_8"],
                                  mt["BVG2_8"], mt["B121_8"], mt["B121N_8"],
                                  mt["B10M1_8"], mt["B10M1X2_8"],
                                  x8h[c], x8l[c], sqx8, sqy8, gxs8, gys8)
                finish_image(84, sqx8, sqy8, gm8)
                # thin8 frame = conv rows 3..83 -> shift 3, 81 rows
                m8 = make_masks(gxs8, gys8, 84, 3, 81, 8)

                ag_in = dpool.tile([81, W], f32, tag="ag_in")
                ag_out = dpool.tile([B * 81, W], f32, tag="ag_out")
                nc.sync.dma_start(out=ag_in[:], in_=gm8[3:84, 1:W + 1])
                nc.gpsimd.collective_compute(
                    "AllGather", Alu.bypass,
                    replica_groups=[list(range(NC))],
                    ins=[ag_in.opt()], outs=[ag_out.opt()])
                a2a_in = dpool.tile([B * 81, W], f32, tag="a2a_in")
                a2a_out = dpool.tile([B * 81, W], f32, tag="a2a_out")
                for b in range(B):
                    dr, dc = DELTAS[b]
                    nc.sync.dma_start(
                        out=a2a_in[81 * b:81 * (b + 1), :],
                        in_=gm8[3 + dr:84 + dr, 1 + dc:W + 1 + dc])
                nc.gpsimd.collective_compute(
                    "AllToAll", Alu.bypass,
                    replica_groups=[list(range(NC))],
                    ins=[a2a_in.opt()], outs=[a2a_out.opt()])
                nc.sync.dma_start(out=ce8[:], in_=gm8[3:84, 1:W + 1])

                # ---- main slab: 8 images x 3 channels
                earlyP = {}

                def emit_early(k, bs):
                    """P[k] for directions bs, computed during the conv tail
                    (images k and k+4 are already done). Both compares on
                    DVE (conv-idle); the and on Pool."""
                    for b in bs:
                        dr, dc = DELTAS[b]
                        nt = 121 if b <= 4 else 120
                        Cs = []
                        for j in (k, k + 4):
                            if b <= 4:
                                i0 = gm_tiles[j][0:nt, 1:W + 1]
                                i1 = (gm_tiles[j][0:nt, 1 + dc:W + 1 + dc]
                                      if dr == 0 else
                                      ce[j][0:nt, 1 + dc:W + 1 + dc])
                            else:
                                i0 = ce[j][0:nt, 1:W + 1]
                                i1 = gm_tiles[j][0:nt, 1 + dc:W + 1 + dc]
                            cj = gmpool.tile([121, W], f16, tag="cE",
                                             name="cE", bufs=2)
                            nc.vector.tensor_tensor(out=cj[0:nt, :], in0=i0,
                                                    in1=i1, op=Alu.is_gt)
                            Cs.append(cj)
                        pk = gmpool.tile([121, W], f16, tag=f"pE{k}_{b}",
                                         name=f"pE{k}_{b}")
                        nc.gpsimd.tensor_tensor(out=pk[0:nt, :],
                                                in0=Cs[0][0:nt, :],
                                                in1=Cs[1][0:nt, :],
                                                op=Alu.mult)
                        earlyP[(b, k)] = pk

                for j in range(B):
                    gm_j = gmpool.tile([122, W + 2], f32, tag=f"gm{j}",
                                       name=f"gm{j}")
                    nc.gpsimd.memset(gm_j[:, 0:1], 0.0)
                    nc.gpsimd.memset(gm_j[:, W + 1:W + 2], 0.0)
                    sqx_st = sqpool.tile([122, 3 * W], f32, tag="sqx",
                                         name="sqx")
                    sqy_st = sqpool.tile([122, 3 * W], f32, tag="sqy",
                                         name="sqy")
                    gxs_sb = gspool.tile([122, W], f32, tag="gxs",
                                         name="gxs")
                    gys_sb = gspool.tile([122, W], f32, tag="gys",
                                         name="gys")
                    for c in range(3):
                        conv_pipeline(c, 128, 124, 122, mt["BV"], mt["BVG1"],
                                      mt["BVG2"], mt["B121"], mt["B121N"],
                                      mt["B10M1"], mt["B10M1X2"],
                                      xh[3 * j + c], xl[3 * j + c],
                                      sqx_st, sqy_st, gxs_sb, gys_sb)
                    finish_image(122, sqx_st, sqy_st, gm_j)
                    gm_tiles.append(gm_j)
                    # dirs 0-4 (shifted frame): masks shift 0, 121 rows;
                    # dirs 5-7 (thin frame): masks shift 1, 120 rows.
                    if j <= 4:
                        masks.append(make_masks(gxs_sb, gys_sb, 122, 0,
                                                121, j))
                    else:
                        masks.append(make_masks(gxs_sb, gys_sb, 122, 1,
                                                120, j))
                    cet = gmpool.tile([121, W + 2], f32, tag=f"ce{j}",
                                      name=f"ce{j}")
                    nc.sync.dma_start(out=cet[:], in_=gm_j[1:122, :])
                    ce.append(cet)
                    # k=0 pair (images 0,4) ready after image 4; split the
                    # emission across two images to avoid head-of-line
                    # stalls of the next image's conv DVE ops.
                    if j == 4:
                        emit_early(0, (0, 1, 2))
                    elif j == 5:
                        emit_early(0, (3, 4, 5))
                    elif j == 6:
                        emit_early(0, (6, 7))
                        emit_early(1, (0,))
                    elif j == 7:
                        emit_early(1, (1, 2, 3, 4, 5, 6, 7))

            # =========== NMS phase ========================================
            # thin frame (dirs 5-7): row p <-> img 118i-1+p, 120 rows.
            # shifted frame (dirs 0-4): row r <-> thin p=r-1, 121 rows.
            with (
                tc.tile_pool(name="cmap", bufs=4) as cpool2,
                tc.tile_pool(name="pmap", bufs=1) as ppool,
                tc.tile_pool(name="g8p", bufs=3) as g8pool,
                tc.tile_pool(name="nmst", bufs=2) as npool,
                tc.tile_pool(name="outp", bufs=2) as opool,
                tc.tile_pool(name="psC", bufs=2, space="PSUM") as psC,
            ):
                def nms_stage1(b_masks, gm_b, get_in0, get_in1, n_thin,
                               bones, n_mp, out_lo, out_dram, n_out,
                               pre=None):
                    """Compares + P-ands (DVE+Pool). Returns state for
                    stage2; two-stage emission keeps the in-order DVE queue
                    fed while Pool computes the ands."""
                    P = []
                    for k in range(4):
                        if pre is not None and k in pre:
                            P.append(pre[k])
                            continue
                        Cs = []
                        for j in (k, k + 4):
                            cj = cpool2.tile([121, W], f16, tag="c", name="c")
                            if k != 1 and j == k + 4:
                                # route through idle Pool: d = a-b (Pool),
                                # C = d > 0 (DVE ts, 2x mode)
                                d = cpool2.tile([121, W], f32, tag="d",
                                                name="d", bufs=2)
                                nc.gpsimd.tensor_tensor(
                                    out=d[0:n_thin, :], in0=get_in0(j),
                                    in1=get_in1(j), op=Alu.subtract)
                                nc.vector.tensor_scalar(
                                    out=cj[0:n_thin, :], in0=d[0:n_thin, :],
                                    scalar1=0.0, scalar2=None, op0=Alu.is_gt)
                            else:
                                nc.vector.tensor_tensor(
                                    out=cj[0:n_thin, :], in0=get_in0(j),
                                    in1=get_in1(j), op=Alu.is_gt)
                            Cs.append(cj)
                        tag = "psel" if k == 3 else f"p{k}"
                        pk = ppool.tile([121, W], f16, tag=tag, name=tag,
                                        bufs=2)
                        # and of {0,1} masks == product (Pool has no
                        # logical ops)
                        nc.gpsimd.tensor_tensor(
                            out=pk[0:n_thin, :], in0=Cs[0][0:n_thin, :],
                            in1=Cs[1][0:n_thin, :], op=Alu.mult)
                        P.append(pk)
                    return (P, b_masks, gm_b, n_thin, bones, n_mp, out_lo,
                            out_dram, n_out)

                def nms_stage2(st):
                    """Select by class masks + thresholds + hysteresis."""
                    (P, b_masks, gm_b, n_thin, bones, n_mp, out_lo,
                     out_dram, n_out) = st
                    c0, c2, sm = b_masks
                    psel = P[3]
                    nc.vector.copy_predicated(out=psel[0:n_thin, :],
                                              mask=sm[0:n_thin, :],
                                              data=P[1][0:n_thin, :])
                    nc.vector.copy_predicated(out=psel[0:n_thin, :],
                                              mask=c0[0:n_thin, :],
                                              data=P[0][0:n_thin, :])
                    nc.vector.copy_predicated(out=psel[0:n_thin, :],
                                              mask=c2[0:n_thin, :],
                                              data=P[2][0:n_thin, :])
                    tq = npool.tile([121, W], f16, tag="tq", name="tq")
                    th = npool.tile([121, W], f16, tag="th", name="th")
                    nc.vector.tensor_scalar(out=tq[0:n_thin, :], in0=gm_b,
                                            scalar1=LOW_S, scalar2=None,
                                            op0=Alu.is_ge)
                    nc.vector.tensor_scalar(out=th[0:n_thin, :], in0=gm_b,
                                            scalar1=HIGH_S, scalar2=None,
                                            op0=Alu.is_gt)
                    q = npool.tile([121, W], f16, tag="q", name="q")
                    nc.vector.tensor_tensor(out=q[0:n_thin, :],
                                            in0=tq[0:n_thin, :],
                                            in1=psel[0:n_thin, :],
                                            op=Alu.logical_and)
                    strong = npool.tile([121, W + 2], f16, tag="strong",
                                        name="strong")
                    nc.gpsimd.memset(strong[:, 0:1], 0.0)
                    nc.gpsimd.memset(strong[:, W + 1:W + 2], 0.0)
                    nc.vector.tensor_tensor(out=strong[0:n_thin, 1:W + 1],
                                            in0=th[0:n_thin, :],
                                            in1=q[0:n_thin, :],
                                            op=Alu.logical_and)
                    # mp = 3x3 box sum of strong: 3 col-shifted matmuls
                    mp = psC.tile([120, W], f32, tag="mp", name="mp")
                    for lo2 in (0, 512):
                        for t in range(3):
                            nc.tensor.matmul(
                                out=mp[0:n_mp, lo2:lo2 + 512],
                                lhsT=bones[0:n_thin, 0:n_mp],
                                rhs=strong[0:n_thin, t + lo2:t + lo2 + 512],
                                start=(t == 0), stop=(t == 2))
                    ot = opool.tile([120, W], f16, tag="ot", name="ot")
                    nc.vector.scalar_tensor_tensor(
                        out=ot[0:n_mp, :], in0=mp[0:n_mp, :], scalar=0.5,
                        in1=q[0:n_mp, :], op0=Alu.is_ge, op1=Alu.logical_and)
                    nc.gpsimd.memset(ot[0:n_mp, 0:1], 0.0)
                    nc.gpsimd.memset(ot[0:n_mp, W - 1:W], 0.0)
                    nc.sync.dma_start(out=out_dram,
                                      in_=ot[out_lo:out_lo + n_out, :])

                def nms_b(b):
                    dr, dc = DELTAS[b]
                    if b <= 4:
                        # shifted frame, 121 rows: in0 = raw gm rows 0..120,
                        # +1-row shift = ce (gm rows 1..121)
                        def in0(j):
                            return gm_tiles[j][0:121, 1:W + 1]

                        def in1(j):
                            if dr == 0:
                                return gm_tiles[j][0:121, 1 + dc:W + 1 + dc]
                            return ce[j][0:121, 1 + dc:W + 1 + dc]

                        return nms_stage1(masks[b],
                                          gm_tiles[b][0:121, 1:W + 1],
                                          in0, in1, 121, mt["BONES_D"], 120,
                                          2, outm[b], SLAB,
                                          pre={k: earlyP[(b, k)]
                                               for k in range(4)
                                               if (b, k) in earlyP})
                    else:
                        # thin frame, 120 rows: in0 = ce rows 0..119
                        # (gm rows 1..120), -1-row shift = raw gm rows 0..119
                        def in0(j):
                            return ce[j][0:120, 1:W + 1]

                        def in1(j):
                            return gm_tiles[j][0:120, 1 + dc:W + 1 + dc]

                        return nms_stage1(masks[b], ce[b][0:120, 1:W + 1],
                                          in0, in1, 120, mt["BONES"], 119,
                                          1, outm[b], SLAB,
                                          pre={k: earlyP[(b, k)]
                                               for k in range(4)
                                               if (b, k) in earlyP})

                # B8: own image only; shifted operands came via AllToAll.
                # thin8 frame: row p (base 0) <-> img 943+p, 81 rows.
                def load8(dram_src, tag):
                    def get(j):
                        t = g8pool.tile([81, W], f32, tag=tag, name=tag,
                                        bufs=2)
                        nc.sync.dma_start(
                            out=t[:], in_=dram_src[81 * j:81 * (j + 1), :])
                        return t[:]
                    return get

                prev = None
                for b in (0, 4, 8, 1, 2, 3, 5, 6, 7):  # 8 = B8 band
                    if b == 8:
                        st = nms_stage1(m8, ce8[:], load8(ag_out, "g8p"),
                                        load8(a2a_out, "g8s"), 81,
                                        mt["BONES8"], 81, 1, out8[:],
                                        B8_ROWS)
                    else:
                        st = nms_b(b)
                    if prev is not None:
                        nms_stage2(prev)
                    prev = st
                nms_stage2(prev)

    _legalize_waits(nc)
    _CACHE["nc"] = nc
    return nc


def _legalize_waits(nc):
    """Several ISA encodings (S2S2D2_STT, HWDGE DMACopy, ...) hold only one
    embedded sync-wait, but Tile's scheduler can attach more. Hoist all
    embedded waits of multi-wait instructions into a NoOp injected just
    before them on the same engine queue (NoOps carry many waits fine)."""
    import concourse.mybir as mybir
    n = 0
    for f in nc.m.functions:
        for blk in f.blocks:
            out = []
            for ins in blk.instructions:
                si = ins.sync_info
                if (si is not None and si.on_wait is not None
                        and len(si.on_wait) > 1):
                    for w in si.on_wait:
                        nop = mybir.InstNoOp(
                            name=f"WFIX-{n}", engine=ins.engine,
                            sync_info=mybir.SyncInfo(on_wait=[w],
                                                     on_update=[]))
                        n += 1
                        out.append(nop)
                    ins.sync_info = mybir.SyncInfo(
                        on_wait=[],
                        on_update=list(si.on_update or []))
                out.append(ins)
            blk.instructions = out


def _in_maps(img):
    img = np.asarray(img, dtype=np.float32)
    hi = img.astype(np.float16)
    lo = (img - hi.astype(np.float32)).astype(np.float16)
    pad = np.zeros((B, 3, 5, W), np.float16)
    hip = np.concatenate([pad, hi], axis=2)  # rows shifted by +5
    lop = np.concatenate([pad, lo], axis=2)
    maps = []
    for i in range(NC):
        r0 = SLAB * i  # padded row index of img row 118i-5
        m = {"xh": np.ascontiguousarray(
                 hip[:, :, r0:r0 + 128, :].reshape(B * 3, 128, W)),
             "xl": np.ascontiguousarray(
                 lop[:, :, r0:r0 + 128, :].reshape(B * 3, 128, W)),
             "x8h": np.ascontiguousarray(hi[i, :, B8_START - 8:, :]),
             "x8l": np.ascontiguousarray(lo[i, :, B8_START - 8:, :])}
        m.update(_const_mats(i))
        maps.append(m)
    return maps


def kernel(img, gauss_h=None, gauss_v=None, sobel_h=None, sobel_v=None,
           dir_f=None, connect_f=None, _want_time=False):
    from concourse.bass_utils import run_bass_kernel_spmd
    nc = _build_program()
    maps = _in_maps(np.asarray(img))
    res = run_bass_kernel_spmd(nc, maps, list(range(NC)), trace=_want_time)
    out = np.zeros((B, 1, H, W), np.float32)
    for i in range(NC):
        r = res.results[i]
        out[:, 0, SLAB * i:SLAB * (i + 1), :] = \
            np.asarray(r["outm"], np.float32)
        out[i, 0, B8_START:, :] = np.asarray(r["out8"], np.float32)
    if _want_time:
        return out, res
    return out


# revision 38
# speedup vs baseline: 1.0005x; 1.0005x over previous
"""Canny edge detector on 8 Trainium2 NeuronCores (Bass/Tile).

Sharding: row slabs. Core i owns output rows [118*i, 118*(i+1)) of ALL 8
images. (The reference's flat gather at B=8 cross-wires images inside NMS:
sel_pos(b,h,w) = dirconv_b(gm_{idx(b,h,w)})(h,w), so every output pixel needs
all 8 images' gradient-magnitude maps at its rows -> shard by rows, not by
image.) The leftover band (rows 944..1023) is computed per-image on the
owning core; gm maps are exchanged through DRAM collectives (AllGather for
plain maps, AllToAll for reader-direction-shifted maps - the per-core shift
must live in data routing because SPMD shares one instruction stream). The
B8 block runs FIRST so both collectives hide under the main-slab conv.

Numerics: the output is a thresholded argmax - iid noise of eps relative
flips ~45k*eps pixels, and the gate (rel 2e-2 ~ 840 flips) needs eps <=
1.5e-5. fp16 or float32r matmuls are far too lossy (measured), BUT fp16
products accumulate exactly into fp32 PSUM, so the conv phase uses
fp16 PAIRS: x = x_hi + x_lo (host-split), bl = sum_h W_h @ {x_hi,x_lo};
blt = blt_hi + blt_lo (Act copy + DVE residual); sobel = exact +-1/2
fp16 weights on both halves. Only systematic error left is the fp16
rounding of the gauss weights, minimized by a host-side scale search
(the scale folds into the LOW/HIGH thresholds; masks are scale-free).
gm / orientation stay fp32; NMS indicator algebra runs in fp16/u8.

SBUF: directions 0-4 run the NMS in a row-shifted frame (tile row r =
thin row p + 1) so their shifted compare operand is the RAW gm tile and
only one aligned copy (ce = gm rows 1..121) is ever made - no dn tiles.

All compute-engine APs must start at partition 0 (HW constraint), so row
re-alignment between pipeline stages is done with SBUF->SBUF DMAs.
"""

import os

os.environ.setdefault("BY_DEFAULT_DISABLE_SUBTILE_DEPS", "1")

import numpy as np

H = 1024
W = 1024
B = 8
NC = 8
SLAB = 118                    # main-slab output rows per core
B8_START = SLAB * NC          # 944
B8_ROWS = H - B8_START        # 80
LOW_T, HIGH_T = 2.5, 5.0
T22SQ = float(np.float32(np.tan(np.pi / 8.0)) ** 2)

DELTAS = {0: (0, 1), 1: (1, 1), 2: (1, 0), 3: (1, -1),
          4: (0, -1), 5: (-1, -1), 6: (-1, 0), 7: (-1, 1)}


def _gauss5():
    n = np.arange(5, dtype=np.float32) - 2.0
    return np.exp(-0.5 * n * n).astype(np.float32)


def _wscale():
    """Scale s minimizing fp16 rounding of the 6 distinct 2D-gauss weights
    s*gi*gj. gm scales by s; the LOW/HIGH thresholds absorb it."""
    if "s" in _CACHE:
        return _CACHE["s"]
    g = _gauss5().astype(np.float64)
    prods = np.array([g[i] * g[j] for i in range(3) for j in range(i, 3)])
    best, bs = 1e9, 1.0
    for s in np.linspace(1.0, 2.0, 65536, endpoint=False):
        r = np.abs(np.float64(np.float16(s * prods)) / (s * prods) - 1.0)
        m = r.max()
        if m < best:
            best, bs = m, float(s)
    _CACHE["s"] = bs
    return bs


def _band(n_in, n_out, offset, taps):
    """M[k, m] = taps[k - m - offset] for k-m-offset in range(len(taps))."""
    m_ = np.zeros((n_in, n_out), np.float32)
    for mm in range(n_out):
        for t, w in enumerate(taps):
            k = mm + offset + t
            if 0 <= k < n_in:
                m_[k, mm] = w
    return m_


def _const_mats(core):
    g = _gauss5()
    s = _wscale()
    sg = [float(np.float16(s * g[0] * gv)) for gv in g]  # pre-rounded row h=0
    mats = {}
    # fused 2D blur: bl = sum_h band(f16(s*g_h*g)) @ x_colshift_h
    for name, gh in (("BV", g[0]), ("BVG1", g[1]), ("BVG2", g[2])):
        taps = [float(np.float16(s * gh * gv)) for gv in g]
        mats[name] = _band(128, 124, 0, taps)
        mats[name + "_8"] = _band(88, 86, 0, taps)
    b121 = _band(124, 122, 0, [1.0, 2.0, 1.0])
    b10m1 = _band(124, 122, 0, [1.0, 0.0, -1.0])
    if core == 0:  # img rows -2,-1 must yield gm=0 (zero-pad semantics)
        b121[:, 0:2] = 0.0
        b10m1[:, 0:2] = 0.0
    mats["B121"] = b121
    mats["B121N"] = -b121
    mats["B10M1"] = b10m1
    mats["B10M1X2"] = 2.0 * b10m1
    # thin frame (dirs 5-7): strong row k <-> thin p=k; mp row p; col0 dummy
    bones = _band(120, 119, -1, [1.0, 1.0, 1.0])
    bones[:, 0] = 0.0
    if core == 0:
        bones[:, 1] = 0.0  # border img row 0
    mats["BONES"] = bones
    # shifted frame (dirs 0-4): strong row k <-> thin p=k-1; mp row m <->
    # thin p=m-1; needs strong k in {m-1,m,m+1}
    bones_d = _band(121, 120, -1, [1.0, 1.0, 1.0])
    bones_d[:, 0:2] = 0.0
    if core == 0:
        bones_d[:, 2] = 0.0  # border img row 0 (thin p=1 -> m=2)
    mats["BONES_D"] = bones_d
    # B8 block: x8 row k <-> img 936+k; bl8 row m <-> img 938+m (86 rows)
    b121_8 = _band(86, 84, 1, [1.0, 2.0, 1.0])
    b10m1_8 = _band(86, 84, 1, [1.0, 0.0, -1.0])
    mats["B121_8"] = b121_8
    mats["B121N_8"] = -b121_8
    mats["B10M1_8"] = b10m1_8
    mats["B10M1X2_8"] = 2.0 * b10m1_8
    bones8 = _band(81, 81, -1, [1.0, 1.0, 1.0])
    bones8[:, 0] = 0.0
    bones8[:, 80] = 0.0  # border row 1023
    mats["BONES8"] = bones8
    return {k: np.ascontiguousarray(v, np.float16) for k, v in mats.items()}


MAT_SPECS = {
    # B8 conv runs first: its matrices load first so PE starts early
    "BV_8": [88, 86], "BVG1_8": [88, 86], "BVG2_8": [88, 86],
    "B121_8": [86, 84], "B121N_8": [86, 84],
    "B10M1_8": [86, 84], "B10M1X2_8": [86, 84],
    "BV": [128, 124], "BVG1": [128, 124], "BVG2": [128, 124],
    "B121": [124, 122], "B121N": [124, 122],
    "B10M1": [124, 122], "B10M1X2": [124, 122],
    "BONES": [120, 119], "BONES_D": [121, 120], "BONES8": [81, 81],
}

_CACHE = {}


def _build_program():
    if "nc" in _CACHE:
        return _CACHE["nc"]
    import concourse.bass as bass
    import concourse.mybir as mybir
    from concourse.tile import TileContext

    f32 = mybir.dt.float32
    f16 = mybir.dt.float16
    u8 = mybir.dt.uint8
    Alu = mybir.AluOpType
    s = _wscale()
    LOW_S, HIGH_S = LOW_T * s, HIGH_T * s

    nc = bass.Bass()

    xh = nc.declare_dram_parameter("xh", [B * 3, 128, W], f16, isOutput=False)
    xl = nc.declare_dram_parameter("xl", [B * 3, 128, W], f16, isOutput=False)
    x8h = nc.declare_dram_parameter("x8h", [3, 88, W], f16, isOutput=False)
    x8l = nc.declare_dram_parameter("x8l", [3, 88, W], f16, isOutput=False)
    mat_d = {k: nc.declare_dram_parameter(k, v, f16, isOutput=False)
             for k, v in MAT_SPECS.items()}
    outm = nc.declare_dram_parameter("outm", [B, SLAB, W], f16, isOutput=True)
    out8 = nc.declare_dram_parameter("out8", [B8_ROWS, W], f16, isOutput=True)

    with TileContext(nc) as tc:
        with (
            tc.tile_pool(name="consts", bufs=1) as cpool,
            tc.tile_pool(name="gmp", bufs=1) as gmpool,
            tc.tile_pool(name="msk", bufs=1) as mskpool,
            tc.tile_pool(name="dram", bufs=1, space="DRAM") as dpool,
        ):
            mt = {}
            for name, shp in MAT_SPECS.items():
                t = cpool.tile(shp, f16, tag=name, name=name)
                nc.sync.dma_start(out=t[:], in_=mat_d[name][:])
                mt[name] = t

            gm_tiles = []
            masks = []
            ce = []
            gm8 = gmpool.tile([85, W + 2], f32, tag="gm8self")
            ce8 = gmpool.tile([81, W], f32, tag="ce8self")
            # =========== conv phase =======================================
            with (
                tc.tile_pool(name="xin", bufs=2) as xpool,
                tc.tile_pool(name="bls", bufs=2) as blspool,
                tc.tile_pool(name="sq", bufs=1) as sqpool,
                tc.tile_pool(name="gsm", bufs=2) as gspool,
                tc.tile_pool(name="mskt", bufs=1) as msktpool,
                tc.tile_pool(name="psA", bufs=2, space="PSUM") as psA,
                tc.tile_pool(name="psB", bufs=2, space="PSUM") as psB,
            ):
                def conv_pipeline(c, n_in, n_bl, n_gxy, bv, bvg1, bvg2,
                                  b121, b121n, b10m1, b10m1x2,
                                  xsrc_h, xsrc_l, sqx_st, sqy_st,
                                  gxs_sb, gys_sb):
                    """One (image, channel): fp16-pair blur + sobel."""
                    xth = xpool.tile([128, W + 4], f16, tag="xh", name="xth")
                    xtl = xpool.tile([128, W + 4], f16, tag="xl", name="xtl")
                    for xt, src in ((xth, xsrc_h), (xtl, xsrc_l)):
                        nc.gpsimd.memset(xt[:, 0:2], 0.0)
                        nc.gpsimd.memset(xt[:, W + 2:W + 4], 0.0)
                        nc.sync.dma_start(out=xt[0:n_in, 2:W + 2], in_=src)
                    bl = psA.tile([124, W], f32, tag="bl", name="bl")
                    lhs5 = [bv, bvg1, bvg2, bvg1, bv]
                    for lo in (0, 512):
                        first = True
                        for xt in (xth, xtl):
                            for h_ in range(5):
                                nc.tensor.matmul(
                                    out=bl[0:n_bl, lo:lo + 512],
                                    lhsT=lhs5[h_][0:n_in, 0:n_bl],
                                    rhs=xt[0:n_in, h_ + lo:h_ + lo + 512],
                                    start=first,
                                    stop=(xt is xtl and h_ == 4))
                                first = False
                    # fp16 pair of bl for the sobel rhs
                    blh = blspool.tile([124, W + 2], f16, tag="blh",
                                       name="blh")
                    bll = blspool.tile([124, W + 2], f16, tag="bll",
                                       name="bll", bufs=1)
                    for t in (blh, bll):
                        nc.gpsimd.memset(t[:, 0:1], 0.0)
                        nc.gpsimd.memset(t[:, W + 1:W + 2], 0.0)
                    nc.scalar.copy(out=blh[0:n_bl, 1:W + 1], in_=bl[0:n_bl, :])
                    nc.vector.tensor_tensor(out=bll[0:n_bl, 1:W + 1],
                                            in0=bl[0:n_bl, :],
                                            in1=blh[0:n_bl, 1:W + 1],
                                            op=Alu.subtract)
                    gx = psB.tile([122, W], f32, tag="gxy", name="gx")
                    gy = psB.tile([122, W], f32, tag="gxy", name="gy")
                    for lo in (0, 512):
                        for i, blt in enumerate((blh, bll)):
                            st = (i == 0)
                            sp = (i == 1)
                            nc.tensor.matmul(
                                out=gx[0:n_gxy, lo:lo + 512],
                                lhsT=b121[0:n_bl, 0:n_gxy],
                                rhs=blt[0:n_bl, lo:lo + 512],
                                start=st, stop=False)
                            nc.tensor.matmul(
                                out=gx[0:n_gxy, lo:lo + 512],
                                lhsT=b121n[0:n_bl, 0:n_gxy],
                                rhs=blt[0:n_bl, 2 + lo:2 + lo + 512],
                                start=False, stop=sp)
                            nc.tensor.matmul(
                                out=gy[0:n_gxy, lo:lo + 512],
                                lhsT=b10m1[0:n_bl, 0:n_gxy],
                                rhs=blt[0:n_bl, 2 + lo:2 + lo + 512],
                                start=st, stop=False)
                            nc.tensor.matmul(
                                out=gy[0:n_gxy, lo:lo + 512],
                                lhsT=b10m1x2[0:n_bl, 0:n_gxy],
                                rhs=blt[0:n_bl, 1 + lo:1 + lo + 512],
                                start=False, stop=False)
                            nc.tensor.matmul(
                                out=gy[0:n_gxy, lo:lo + 512],
                                lhsT=b10m1[0:n_bl, 0:n_gxy],
                                rhs=blt[0:n_bl, lo:lo + 512],
                                start=False, stop=sp)
                    nc.scalar.square(out=sqx_st[0:n_gxy, c * W:(c + 1) * W],
                                     in_=gx[0:n_gxy, :])
                    nc.scalar.square(out=sqy_st[0:n_gxy, c * W:(c + 1) * W],
                                     in_=gy[0:n_gxy, :])
                    # gxs/gys accumulation in f32 SBUF (masks need f32)
                    if c == 0:
                        nc.scalar.copy(out=gxs_sb[0:n_gxy, :],
                                       in_=gx[0:n_gxy, :])
                        nc.scalar.copy(out=gys_sb[0:n_gxy, :],
                                       in_=gy[0:n_gxy, :])
                    else:
                        nc.vector.tensor_tensor(out=gxs_sb[0:n_gxy, :],
                                                in0=gxs_sb[0:n_gxy, :],
                                                in1=gx[0:n_gxy, :],
                                                op=Alu.add)
                        nc.vector.tensor_tensor(out=gys_sb[0:n_gxy, :],
                                                in0=gys_sb[0:n_gxy, :],
                                                in1=gy[0:n_gxy, :],
                                                op=Alu.add)

                def finish_image(n_gxy, sqx_st, sqy_st, gm_t):
                    """magnitude: m2 (Pool), sqrt (Act), gm chunk adds."""
                    nc.gpsimd.tensor_tensor(out=sqx_st[0:n_gxy, :],
                                            in0=sqx_st[0:n_gxy, :],
                                            in1=sqy_st[0:n_gxy, :],
                                            op=Alu.add)
                    # reuse sqy's buffer: m2 (its last reader) just finished
                    mag = sqpool.tile([122, 3 * W], f32, tag="sqy",
                                      name="mag")
                    nc.scalar.sqrt(out=mag[0:n_gxy, :], in_=sqx_st[0:n_gxy, :])
                    gmi = gm_t[0:n_gxy, 1:W + 1]
                    nc.vector.tensor_tensor(out=gmi, in0=mag[0:n_gxy, 0:W],
                                            in1=mag[0:n_gxy, W:2 * W],
                                            op=Alu.add)
                    nc.vector.tensor_tensor(out=gmi, in0=gmi,
                                            in1=mag[0:n_gxy, 2 * W:3 * W],
                                            op=Alu.add)

                def make_masks(gxs_sb, gys_sb, n, shift, n_thin, j):
                    """u8 class masks at conv frame [0:n], DMA-shifted down
                    by `shift` rows into persistent thin-frame tiles."""
                    a2 = gspool.tile([122, W], f32, tag="a2", name="a2",
                                     bufs=1)
                    b2 = gspool.tile([122, W], f32, tag="b2", name="b2",
                                     bufs=1)
                    nc.scalar.square(out=a2[0:n, :], in_=gxs_sb[0:n, :])
                    nc.scalar.square(out=b2[0:n, :], in_=gys_sb[0:n, :])
                    sgx = gspool.tile([122, W], u8, tag="sgx", name="sgx",
                                      bufs=1)
                    sgy = gspool.tile([122, W], u8, tag="sgy", name="sgy",
                                      bufs=1)
                    nc.vector.tensor_scalar(out=sgx[0:n, :],
                                            in0=gxs_sb[0:n, :], scalar1=0.0,
                                            scalar2=None, op0=Alu.is_ge)
                    nc.vector.tensor_scalar(out=sgy[0:n, :],
                                            in0=gys_sb[0:n, :], scalar1=0.0,
                                            scalar2=None, op0=Alu.is_ge)
                    tmp = [msktpool.tile([122, W], u8, tag=t, name=t)
                           for t in ("tc0", "tc2", "tsm")]
                    nc.vector.scalar_tensor_tensor(
                        out=tmp[0][0:n, :], in0=a2[0:n, :], scalar=T22SQ,
                        in1=b2[0:n, :], op0=Alu.mult, op1=Alu.is_gt)
                    nc.vector.scalar_tensor_tensor(
                        out=tmp[1][0:n, :], in0=b2[0:n, :], scalar=T22SQ,
                        in1=a2[0:n, :], op0=Alu.mult, op1=Alu.is_gt)
                    # sign agreement == (ab >= 0) wherever c0/c2 don't apply
                    nc.vector.tensor_tensor(out=tmp[2][0:n, :],
                                            in0=sgx[0:n, :], in1=sgy[0:n, :],
                                            op=Alu.is_equal)
                    out = []
                    for t, tag in zip(tmp, ("c0", "c2", "sm")):
                        p = mskpool.tile([n_thin, W], u8, tag=f"{tag}_{j}",
                                         name=f"{tag}_{j}")
                        nc.sync.dma_start(out=p[:],
                                          in_=t[shift:shift + n_thin, :])
                        out.append(p)
                    return out

                # ---- B8 block FIRST so the collectives hide under main conv
                nc.vector.memset(gm8[:], 0.0)
                sqx8 = sqpool.tile([122, 3 * W], f32, tag="sqx", name="sqx")
                sqy8 = sqpool.tile([122, 3 * W], f32, tag="sqy", name="sqy")
                gxs8 = gspool.tile([122, W], f32, tag="gxs", name="gxs")
                gys8 = gspool.tile([122, W], f32, tag="gys", name="gys")
                for c in range(3):
                    conv_pipeline(c, 88, 86, 84, mt["BV_8"], mt["BVG1_8"],
                                  mt["BVG2_8"], mt["B121_8"], mt["B121N_8"],
                                  mt["B10M1_8"], mt["B10M1X2_8"],
                                  x8h[c], x8l[c], sqx8, sqy8, gxs8, gys8)
                finish_image(84, sqx8, sqy8, gm8)
                # thin8 frame = conv rows 3..83 -> shift 3, 81 rows
                m8 = make_masks(gxs8, gys8, 84, 3, 81, 8)

                ag_in = dpool.tile([81, W], f32, tag="ag_in")
                ag_out = dpool.tile([B * 81, W], f32, tag="ag_out")
                nc.sync.dma_start(out=ag_in[:], in_=gm8[3:84, 1:W + 1])
                nc.gpsimd.collective_compute(
                    "AllGather", Alu.bypass,
                    replica_groups=[list(range(NC))],
                    ins=[ag_in.opt()], outs=[ag_out.opt()])
                a2a_in = dpool.tile([B * 81, W], f32, tag="a2a_in")
                a2a_out = dpool.tile([B * 81, W], f32, tag="a2a_out")
                for b in range(B):
                    dr, dc = DELTAS[b]
                    nc.sync.dma_start(
                        out=a2a_in[81 * b:81 * (b + 1), :],
                        in_=gm8[3 + dr:84 + dr, 1 + dc:W + 1 + dc])
                nc.gpsimd.collective_compute(
                    "AllToAll", Alu.bypass,
                    replica_groups=[list(range(NC))],
                    ins=[a2a_in.opt()], outs=[a2a_out.opt()])
                nc.sync.dma_start(out=ce8[:], in_=gm8[3:84, 1:W + 1])

                # ---- main slab: 8 images x 3 channels
                earlyP = {}

                def emit_early(k, bs):
                    """P[k] for directions bs, computed during the conv tail
                    (images k and k+4 are already done). Both compares on
                    DVE (conv-idle); the and on Pool."""
                    for b in bs:
                        dr, dc = DELTAS[b]
                        nt = 121 if b <= 4 else 120
                        Cs = []
                        for j in (k, k + 4):
                            if b <= 4:
                                i0 = gm_tiles[j][0:nt, 1:W + 1]
                                i1 = (gm_tiles[j][0:nt, 1 + dc:W + 1 + dc]
                                      if dr == 0 else
                                      ce[j][0:nt, 1 + dc:W + 1 + dc])
                            else:
                                i0 = ce[j][0:nt, 1:W + 1]
                                i1 = gm_tiles[j][0:nt, 1 + dc:W + 1 + dc]
                            cj = gmpool.tile([121, W], f16, tag="cE",
                                             name="cE", bufs=2)
                            nc.vector.tensor_tensor(out=cj[0:nt, :], in0=i0,
                                                    in1=i1, op=Alu.is_gt)
                            Cs.append(cj)
                        pk = gmpool.tile([121, W], f16, tag=f"pE{k}_{b}",
                                         name=f"pE{k}_{b}")
                        nc.gpsimd.tensor_tensor(out=pk[0:nt, :],
                                                in0=Cs[0][0:nt, :],
                                                in1=Cs[1][0:nt, :],
                                                op=Alu.mult)
                        earlyP[(b, k)] = pk

                for j in range(B):
                    gm_j = gmpool.tile([122, W + 2], f32, tag=f"gm{j}",
                                       name=f"gm{j}")
                    nc.gpsimd.memset(gm_j[:, 0:1], 0.0)
                    nc.gpsimd.memset(gm_j[:, W + 1:W + 2], 0.0)
                    sqx_st = sqpool.tile([122, 3 * W], f32, tag="sqx",
                                         name="sqx")
                    sqy_st = sqpool.tile([122, 3 * W], f32, tag="sqy",
                                         name="sqy")
                    gxs_sb = gspool.tile([122, W], f32, tag="gxs",
                                         name="gxs")
                    gys_sb = gspool.tile([122, W], f32, tag="gys",
                                         name="gys")
                    for c in range(3):
                        conv_pipeline(c, 128, 124, 122, mt["BV"], mt["BVG1"],
                                      mt["BVG2"], mt["B121"], mt["B121N"],
                                      mt["B10M1"], mt["B10M1X2"],
                                      xh[3 * j + c], xl[3 * j + c],
                                      sqx_st, sqy_st, gxs_sb, gys_sb)
                    finish_image(122, sqx_st, sqy_st, gm_j)
                    gm_tiles.append(gm_j)
                    # dirs 0-4 (shifted frame): masks shift 0, 121 rows;
                    # dirs 5-7 (thin frame): masks shift 1, 120 rows.
                    if j <= 4:
                        masks.append(make_masks(gxs_sb, gys_sb, 122, 0,
                                                121, j))
                    else:
                        masks.append(make_masks(gxs_sb, gys_sb, 122, 1,
                                                120, j))
                    cet = gmpool.tile([121, W + 2], f32, tag=f"ce{j}",
                                      name=f"ce{j}")
                    nc.sync.dma_start(out=cet[:], in_=gm_j[1:122, :])
                    ce.append(cet)
                    # k=0 pair (images 0,4) ready after image 4; split the
                    # emission across two images to avoid head-of-line
                    # stalls of the next image's conv DVE ops.
                    if j == 4:
                        emit_early(0, (0, 1, 2))
                    elif j == 5:
                        emit_early(0, (3, 4, 5))
                    elif j == 6:
                        emit_early(0, (6, 7))
                        emit_early(1, (0,))
                    elif j == 7:
                        emit_early(1, (1, 2, 3, 4, 5, 6, 7))

            # =========== NMS phase ========================================
            # thin frame (dirs 5-7): row p <-> img 118i-1+p, 120 rows.
            # shifted frame (dirs 0-4): row r <-> thin p=r-1, 121 rows.
            with (
                tc.tile_pool(name="cmap", bufs=4) as cpool2,
                tc.tile_pool(name="pmap", bufs=1) as ppool,
                tc.tile_pool(name="g8p", bufs=3) as g8pool,
                tc.tile_pool(name="nmst", bufs=2) as npool,
                tc.tile_pool(name="outp", bufs=2) as opool,
                tc.tile_pool(name="psC", bufs=2, space="PSUM") as psC,
            ):
                def nms_stage1(b_masks, gm_b, get_in0, get_in1, n_thin,
                               bones, n_mp, out_lo, out_dram, n_out,
                               pre=None):
                    """Compares + P-ands (DVE+Pool). Returns state for
                    stage2; two-stage emission keeps the in-order DVE queue
                    fed while Pool computes the ands."""
                    P = []
                    for k in range(4):
                        if pre is not None and k in pre:
                            P.append(pre[k])
                            continue
                        Cs = []
                        for j in (k, k + 4):
                            cj = cpool2.tile([121, W], f16, tag="c", name="c")
                            if k != 1 and j == k + 4:
                                # route through idle Pool: d = a-b (Pool),
                                # C = d > 0 (DVE ts, 2x mode)
                                d = cpool2.tile([121, W], f32, tag="d",
                                                name="d", bufs=2)
                                nc.gpsimd.tensor_tensor(
                                    out=d[0:n_thin, :], in0=get_in0(j),
                                    in1=get_in1(j), op=Alu.subtract)
                                nc.vector.tensor_scalar(
                                    out=cj[0:n_thin, :], in0=d[0:n_thin, :],
                                    scalar1=0.0, scalar2=None, op0=Alu.is_gt)
                            else:
                                nc.vector.tensor_tensor(
                                    out=cj[0:n_thin, :], in0=get_in0(j),
                                    in1=get_in1(j), op=Alu.is_gt)
                            Cs.append(cj)
                        tag = "psel" if k == 3 else f"p{k}"
                        pk = ppool.tile([121, W], f16, tag=tag, name=tag,
                                        bufs=2)
                        # and of {0,1} masks == product (Pool has no
                        # logical ops)
                        nc.gpsimd.tensor_tensor(
                            out=pk[0:n_thin, :], in0=Cs[0][0:n_thin, :],
                            in1=Cs[1][0:n_thin, :], op=Alu.mult)
                        P.append(pk)
                    return (P, b_masks, gm_b, n_thin, bones, n_mp, out_lo,
                            out_dram, n_out)

                def nms_stage2(st):
                    """Select by class masks + thresholds + hysteresis."""
                    (P, b_masks, gm_b, n_thin, bones, n_mp, out_lo,
                     out_dram, n_out) = st
                    c0, c2, sm = b_masks
                    psel = P[3]
                    nc.vector.copy_predicated(out=psel[0:n_thin, :],
                                              mask=sm[0:n_thin, :],
                                              data=P[1][0:n_thin, :])
                    nc.vector.copy_predicated(out=psel[0:n_thin, :],
                                              mask=c0[0:n_thin, :],
                                              data=P[0][0:n_thin, :])
                    nc.vector.copy_predicated(out=psel[0:n_thin, :],
                                              mask=c2[0:n_thin, :],
                                              data=P[2][0:n_thin, :])
                    tq = npool.tile([121, W], f16, tag="tq", name="tq")
                    th = npool.tile([121, W], f16, tag="th", name="th")
                    nc.vector.tensor_scalar(out=tq[0:n_thin, :], in0=gm_b,
                                            scalar1=LOW_S, scalar2=None,
                                            op0=Alu.is_ge)
                    nc.vector.tensor_scalar(out=th[0:n_thin, :], in0=gm_b,
                                            scalar1=HIGH_S, scalar2=None,
                                            op0=Alu.is_gt)
                    q = npool.tile([121, W], f16, tag="q", name="q")
                    nc.vector.tensor_tensor(out=q[0:n_thin, :],
                                            in0=tq[0:n_thin, :],
                                            in1=psel[0:n_thin, :],
                                            op=Alu.logical_and)
                    strong = npool.tile([121, W + 2], f16, tag="strong",
                                        name="strong")
                    nc.gpsimd.memset(strong[:, 0:1], 0.0)
                    nc.gpsimd.memset(strong[:, W + 1:W + 2], 0.0)
                    nc.vector.tensor_tensor(out=strong[0:n_thin, 1:W + 1],
                                            in0=th[0:n_thin, :],
                                            in1=q[0:n_thin, :],
                                            op=Alu.logical_and)
                    # mp = 3x3 box sum of strong: 3 col-shifted matmuls
                    mp = psC.tile([120, W], f32, tag="mp", name="mp")
                    for lo2 in (0, 512):
                        for t in range(3):
                            nc.tensor.matmul(
                                out=mp[0:n_mp, lo2:lo2 + 512],
                                lhsT=bones[0:n_thin, 0:n_mp],
                                rhs=strong[0:n_thin, t + lo2:t + lo2 + 512],
                                start=(t == 0), stop=(t == 2))
                    ot = opool.tile([120, W], f16, tag="ot", name="ot")
                    nc.vector.scalar_tensor_tensor(
                        out=ot[0:n_mp, :], in0=mp[0:n_mp, :], scalar=0.5,
                        in1=q[0:n_mp, :], op0=Alu.is_ge, op1=Alu.logical_and)
                    nc.gpsimd.memset(ot[0:n_mp, 0:1], 0.0)
                    nc.gpsimd.memset(ot[0:n_mp, W - 1:W], 0.0)
                    nc.sync.dma_start(out=out_dram,
                                      in_=ot[out_lo:out_lo + n_out, :])

                def nms_b(b):
                    dr, dc = DELTAS[b]
                    if b <= 4:
                        # shifted frame, 121 rows: in0 = raw gm rows 0..120,
                        # +1-row shift = ce (gm rows 1..121)
                        def in0(j):
                            return gm_tiles[j][0:121, 1:W + 1]

                        def in1(j):
                            if dr == 0:
                                return gm_tiles[j][0:121, 1 + dc:W + 1 + dc]
                            return ce[j][0:121, 1 + dc:W + 1 + dc]

                        return nms_stage1(masks[b],
                                          gm_tiles[b][0:121, 1:W + 1],
                                          in0, in1, 121, mt["BONES_D"], 120,
                                          2, outm[b], SLAB,
                                          pre={k: earlyP[(b, k)]
                                               for k in range(4)
                                               if (b, k) in earlyP})
                    else:
                        # thin frame, 120 rows: in0 = ce rows 0..119
                        # (gm rows 1..120), -1-row shift = raw gm rows 0..119
                        def in0(j):
                            return ce[j][0:120, 1:W + 1]

                        def in1(j):
                            return gm_tiles[j][0:120, 1 + dc:W + 1 + dc]

                        return nms_stage1(masks[b], ce[b][0:120, 1:W + 1],
                                          in0, in1, 120, mt["BONES"], 119,
                                          1, outm[b], SLAB,
                                          pre={k: earlyP[(b, k)]
                                               for k in range(4)
                                               if (b, k) in earlyP})

                # B8: own image only; shifted operands came via AllToAll.
                # thin8 frame: row p (base 0) <-> img 943+p, 81 rows.
                def load8(dram_src, tag):
                    def get(j):
                        t = g8pool.tile([81, W], f32, tag=tag, name=tag,
                                        bufs=2)
                        nc.sync.dma_start(
                            out=t[:], in_=dram_src[81 * j:81 * (j + 1), :])
                        return t[:]
                    return get

                prev = None
                for b in (0, 4, 1, 2, 3, 5, 6, 7, 8):  # 8 = B8 band
                    if b == 8:
                        st = nms_stage1(m8, ce8[:], load8(ag_out, "g8p"),
                                        load8(a2a_out, "g8s"), 81,
                                        mt["BONES8"], 81, 1, out8[:],
                                        B8_ROWS)
                    else:
                        st = nms_b(b)
                    if prev is not None:
                        nms_stage2(prev)
                    prev = st
                nms_stage2(prev)

    _legalize_waits(nc)
    _CACHE["nc"] = nc
    return nc


def _legalize_waits(nc):
    """Several ISA encodings (S2S2D2_STT, HWDGE DMACopy, ...) hold only one
    embedded sync-wait, but Tile's scheduler can attach more. Hoist all
    embedded waits of multi-wait instructions into a NoOp injected just
    before them on the same engine queue (NoOps carry many waits fine)."""
    import concourse.mybir as mybir
    n = 0
    for f in nc.m.functions:
        for blk in f.blocks:
            out = []
            for ins in blk.instructions:
                si = ins.sync_info
                if (si is not None and si.on_wait is not None
                        and len(si.on_wait) > 1):
                    for w in si.on_wait:
                        nop = mybir.InstNoOp(
                            name=f"WFIX-{n}", engine=ins.engine,
                            sync_info=mybir.SyncInfo(on_wait=[w],
                                                     on_update=[]))
                        n += 1
                        out.append(nop)
                    ins.sync_info = mybir.SyncInfo(
                        on_wait=[],
                        on_update=list(si.on_update or []))
                out.append(ins)
            blk.instructions = out


def _in_maps(img):
    img = np.asarray(img, dtype=np.float32)
    hi = img.astype(np.float16)
    lo = (img - hi.astype(np.float32)).astype(np.float16)
    pad = np.zeros((B, 3, 5, W), np.float16)
    hip = np.concatenate([pad, hi], axis=2)  # rows shifted by +5
    lop = np.concatenate([pad, lo], axis=2)
    maps = []
    for i in range(NC):
        r0 = SLAB * i  # padded row index of img row 118i-5
        m = {"xh": np.ascontiguousarray(
                 hip[:, :, r0:r0 + 128, :].reshape(B * 3, 128, W)),
             "xl": np.ascontiguousarray(
                 lop[:, :, r0:r0 + 128, :].reshape(B * 3, 128, W)),
             "x8h": np.ascontiguousarray(hi[i, :, B8_START - 8:, :]),
             "x8l": np.ascontiguousarray(lo[i, :, B8_START - 8:, :])}
        m.update(_const_mats(i))
        maps.append(m)
    return maps


def kernel(img, gauss_h=None, gauss_v=None, sobel_h=None, sobel_v=None,
           dir_f=None, connect_f=None, _want_time=False):
    from concourse.bass_utils import run_bass_kernel_spmd
    nc = _build_program()
    maps = _in_maps(np.asarray(img))
    res = run_bass_kernel_spmd(nc, maps, list(range(NC)), trace=_want_time)
    out = np.zeros((B, 1, H, W), np.float32)
    for i in range(NC):
        r = res.results[i]
        out[:, 0, SLAB * i:SLAB * (i + 1), :] = \
            np.asarray(r["outm"], np.float32)
        out[i, 0, B8_START:, :] = np.asarray(r["out8"], np.float32)
    if _want_time:
        return out, res
    return out


# revision 39
# speedup vs baseline: 1.0035x; 1.0030x over previous
"""Canny edge detector on 8 Trainium2 NeuronCores (Bass/Tile).

Sharding: row slabs. Core i owns output rows [118*i, 118*(i+1)) of ALL 8
images. (The reference's flat gather at B=8 cross-wires images inside NMS:
sel_pos(b,h,w) = dirconv_b(gm_{idx(b,h,w)})(h,w), so every output pixel needs
all 8 images' gradient-magnitude maps at its rows -> shard by rows, not by
image.) The leftover band (rows 944..1023) is computed per-image on the
owning core; gm maps are exchanged through DRAM collectives (AllGather for
plain maps, AllToAll for reader-direction-shifted maps - the per-core shift
must live in data routing because SPMD shares one instruction stream). The
B8 block runs FIRST so both collectives hide under the main-slab conv.

Numerics: the output is a thresholded argmax - iid noise of eps relative
flips ~45k*eps pixels, and the gate (rel 2e-2 ~ 840 flips) needs eps <=
1.5e-5. fp16 or float32r matmuls are far too lossy (measured), BUT fp16
products accumulate exactly into fp32 PSUM, so the conv phase uses
fp16 PAIRS: x = x_hi + x_lo (host-split), bl = sum_h W_h @ {x_hi,x_lo};
blt = blt_hi + blt_lo (Act copy + DVE residual); sobel = exact +-1/2
fp16 weights on both halves. Only systematic error left is the fp16
rounding of the gauss weights, minimized by a host-side scale search
(the scale folds into the LOW/HIGH thresholds; masks are scale-free).
gm / orientation stay fp32; NMS indicator algebra runs in fp16/u8.

SBUF: directions 0-4 run the NMS in a row-shifted frame (tile row r =
thin row p + 1) so their shifted compare operand is the RAW gm tile and
only one aligned copy (ce = gm rows 1..121) is ever made - no dn tiles.

All compute-engine APs must start at partition 0 (HW constraint), so row
re-alignment between pipeline stages is done with SBUF->SBUF DMAs.
"""

import os

os.environ.setdefault("BY_DEFAULT_DISABLE_SUBTILE_DEPS", "1")

import numpy as np

H = 1024
W = 1024
B = 8
NC = 8
SLAB = 118                    # main-slab output rows per core
B8_START = SLAB * NC          # 944
B8_ROWS = H - B8_START        # 80
LOW_T, HIGH_T = 2.5, 5.0
T22SQ = float(np.float32(np.tan(np.pi / 8.0)) ** 2)

DELTAS = {0: (0, 1), 1: (1, 1), 2: (1, 0), 3: (1, -1),
          4: (0, -1), 5: (-1, -1), 6: (-1, 0), 7: (-1, 1)}


def _gauss5():
    n = np.arange(5, dtype=np.float32) - 2.0
    return np.exp(-0.5 * n * n).astype(np.float32)


def _wscale():
    """Scale s minimizing fp16 rounding of the 6 distinct 2D-gauss weights
    s*gi*gj. gm scales by s; the LOW/HIGH thresholds absorb it."""
    if "s" in _CACHE:
        return _CACHE["s"]
    g = _gauss5().astype(np.float64)
    prods = np.array([g[i] * g[j] for i in range(3) for j in range(i, 3)])
    best, bs = 1e9, 1.0
    for s in np.linspace(1.0, 2.0, 65536, endpoint=False):
        r = np.abs(np.float64(np.float16(s * prods)) / (s * prods) - 1.0)
        m = r.max()
        if m < best:
            best, bs = m, float(s)
    _CACHE["s"] = bs
    return bs


def _band(n_in, n_out, offset, taps):
    """M[k, m] = taps[k - m - offset] for k-m-offset in range(len(taps))."""
    m_ = np.zeros((n_in, n_out), np.float32)
    for mm in range(n_out):
        for t, w in enumerate(taps):
            k = mm + offset + t
            if 0 <= k < n_in:
                m_[k, mm] = w
    return m_


def _const_mats(core):
    g = _gauss5()
    s = _wscale()
    sg = [float(np.float16(s * g[0] * gv)) for gv in g]  # pre-rounded row h=0
    mats = {}
    # fused 2D blur: bl = sum_h band(f16(s*g_h*g)) @ x_colshift_h
    for name, gh in (("BV", g[0]), ("BVG1", g[1]), ("BVG2", g[2])):
        taps = [float(np.float16(s * gh * gv)) for gv in g]
        mats[name] = _band(128, 124, 0, taps)
        mats[name + "_8"] = _band(88, 86, 0, taps)
    b121 = _band(124, 122, 0, [1.0, 2.0, 1.0])
    b10m1 = _band(124, 122, 0, [1.0, 0.0, -1.0])
    if core == 0:  # img rows -2,-1 must yield gm=0 (zero-pad semantics)
        b121[:, 0:2] = 0.0
        b10m1[:, 0:2] = 0.0
    mats["B121"] = b121
    mats["B121N"] = -b121
    mats["B10M1"] = b10m1
    mats["B10M1X2"] = 2.0 * b10m1
    # thin frame (dirs 5-7): strong row k <-> thin p=k; mp row p; col0 dummy
    bones = _band(120, 119, -1, [1.0, 1.0, 1.0])
    bones[:, 0] = 0.0
    if core == 0:
        bones[:, 1] = 0.0  # border img row 0
    mats["BONES"] = bones
    # shifted frame (dirs 0-4): strong row k <-> thin p=k-1; mp row m <->
    # thin p=m-1; needs strong k in {m-1,m,m+1}
    bones_d = _band(121, 120, -1, [1.0, 1.0, 1.0])
    bones_d[:, 0:2] = 0.0
    if core == 0:
        bones_d[:, 2] = 0.0  # border img row 0 (thin p=1 -> m=2)
    mats["BONES_D"] = bones_d
    # B8 block: x8 row k <-> img 936+k; bl8 row m <-> img 938+m (86 rows)
    b121_8 = _band(86, 84, 1, [1.0, 2.0, 1.0])
    b10m1_8 = _band(86, 84, 1, [1.0, 0.0, -1.0])
    mats["B121_8"] = b121_8
    mats["B121N_8"] = -b121_8
    mats["B10M1_8"] = b10m1_8
    mats["B10M1X2_8"] = 2.0 * b10m1_8
    bones8 = _band(81, 81, -1, [1.0, 1.0, 1.0])
    bones8[:, 0] = 0.0
    bones8[:, 80] = 0.0  # border row 1023
    mats["BONES8"] = bones8
    return {k: np.ascontiguousarray(v, np.float16) for k, v in mats.items()}


MAT_SPECS = {
    # B8 conv runs first: its matrices load first so PE starts early
    "BV_8": [88, 86], "BVG1_8": [88, 86], "BVG2_8": [88, 86],
    "B121_8": [86, 84], "B121N_8": [86, 84],
    "B10M1_8": [86, 84], "B10M1X2_8": [86, 84],
    "BV": [128, 124], "BVG1": [128, 124], "BVG2": [128, 124],
    "B121": [124, 122], "B121N": [124, 122],
    "B10M1": [124, 122], "B10M1X2": [124, 122],
    "BONES": [120, 119], "BONES_D": [121, 120], "BONES8": [81, 81],
}

_CACHE = {}


def _build_program():
    if "nc" in _CACHE:
        return _CACHE["nc"]
    import concourse.bass as bass
    import concourse.mybir as mybir
    from concourse.tile import TileContext

    f32 = mybir.dt.float32
    f16 = mybir.dt.float16
    u8 = mybir.dt.uint8
    Alu = mybir.AluOpType
    s = _wscale()
    LOW_S, HIGH_S = LOW_T * s, HIGH_T * s

    nc = bass.Bass()

    xh = nc.declare_dram_parameter("xh", [B * 3, 128, W], f16, isOutput=False)
    xl = nc.declare_dram_parameter("xl", [B * 3, 128, W], f16, isOutput=False)
    x8h = nc.declare_dram_parameter("x8h", [3, 88, W], f16, isOutput=False)
    x8l = nc.declare_dram_parameter("x8l", [3, 88, W], f16, isOutput=False)
    mat_d = {k: nc.declare_dram_parameter(k, v, f16, isOutput=False)
             for k, v in MAT_SPECS.items()}
    outm = nc.declare_dram_parameter("outm", [B, SLAB, W], f16, isOutput=True)
    out8 = nc.declare_dram_parameter("out8", [B8_ROWS, W], f16, isOutput=True)

    with TileContext(nc) as tc:
        with (
            tc.tile_pool(name="consts", bufs=1) as cpool,
            tc.tile_pool(name="gmp", bufs=1) as gmpool,
            tc.tile_pool(name="msk", bufs=1) as mskpool,
            tc.tile_pool(name="dram", bufs=1, space="DRAM") as dpool,
        ):
            mt = {}
            for name, shp in MAT_SPECS.items():
                t = cpool.tile(shp, f16, tag=name, name=name)
                nc.sync.dma_start(out=t[:], in_=mat_d[name][:])
                mt[name] = t

            gm_tiles = []
            masks = []
            ce = []
            gm8 = gmpool.tile([85, W + 2], f32, tag="gm8self")
            ce8 = gmpool.tile([81, W], f32, tag="ce8self")
            # =========== conv phase =======================================
            with (
                tc.tile_pool(name="xin", bufs=2) as xpool,
                tc.tile_pool(name="bls", bufs=2) as blspool,
                tc.tile_pool(name="sq", bufs=1) as sqpool,
                tc.tile_pool(name="gsm", bufs=2) as gspool,
                tc.tile_pool(name="mskt", bufs=1) as msktpool,
                tc.tile_pool(name="psA", bufs=2, space="PSUM") as psA,
                tc.tile_pool(name="psB", bufs=2, space="PSUM") as psB,
            ):
                def conv_pipeline(c, n_in, n_bl, n_gxy, bv, bvg1, bvg2,
                                  b121, b121n, b10m1, b10m1x2,
                                  xsrc_h, xsrc_l, sqx_st, sqy_st,
                                  gxs_sb, gys_sb):
                    """One (image, channel): fp16-pair blur + sobel."""
                    xth = xpool.tile([128, W + 4], f16, tag="xh", name="xth")
                    xtl = xpool.tile([128, W + 4], f16, tag="xl", name="xtl")
                    for xt, src in ((xth, xsrc_h), (xtl, xsrc_l)):
                        nc.gpsimd.memset(xt[:, 0:2], 0.0)
                        nc.gpsimd.memset(xt[:, W + 2:W + 4], 0.0)
                        nc.sync.dma_start(out=xt[0:n_in, 2:W + 2], in_=src)
                    bl = psA.tile([124, W], f32, tag="bl", name="bl")
                    lhs5 = [bv, bvg1, bvg2, bvg1, bv]
                    for lo in (0, 512):
                        first = True
                        for xt in (xth, xtl):
                            for h_ in range(5):
                                nc.tensor.matmul(
                                    out=bl[0:n_bl, lo:lo + 512],
                                    lhsT=lhs5[h_][0:n_in, 0:n_bl],
                                    rhs=xt[0:n_in, h_ + lo:h_ + lo + 512],
                                    start=first,
                                    stop=(xt is xtl and h_ == 4))
                                first = False
                    # fp16 pair of bl for the sobel rhs
                    blh = blspool.tile([124, W + 2], f16, tag="blh",
                                       name="blh")
                    bll = blspool.tile([124, W + 2], f16, tag="bll",
                                       name="bll", bufs=1)
                    for t in (blh, bll):
                        nc.gpsimd.memset(t[:, 0:1], 0.0)
                        nc.gpsimd.memset(t[:, W + 1:W + 2], 0.0)
                    nc.scalar.copy(out=blh[0:n_bl, 1:W + 1], in_=bl[0:n_bl, :])
                    nc.vector.tensor_tensor(out=bll[0:n_bl, 1:W + 1],
                                            in0=bl[0:n_bl, :],
                                            in1=blh[0:n_bl, 1:W + 1],
                                            op=Alu.subtract)
                    gx = psB.tile([122, W], f32, tag="gxy", name="gx")
                    gy = psB.tile([122, W], f32, tag="gxy", name="gy")
                    for lo in (0, 512):
                        for i, blt in enumerate((blh, bll)):
                            st = (i == 0)
                            sp = (i == 1)
                            nc.tensor.matmul(
                                out=gx[0:n_gxy, lo:lo + 512],
                                lhsT=b121[0:n_bl, 0:n_gxy],
                                rhs=blt[0:n_bl, lo:lo + 512],
                                start=st, stop=False)
                            nc.tensor.matmul(
                                out=gx[0:n_gxy, lo:lo + 512],
                                lhsT=b121n[0:n_bl, 0:n_gxy],
                                rhs=blt[0:n_bl, 2 + lo:2 + lo + 512],
                                start=False, stop=sp)
                            nc.tensor.matmul(
                                out=gy[0:n_gxy, lo:lo + 512],
                                lhsT=b10m1[0:n_bl, 0:n_gxy],
                                rhs=blt[0:n_bl, 2 + lo:2 + lo + 512],
                                start=st, stop=False)
                            nc.tensor.matmul(
                                out=gy[0:n_gxy, lo:lo + 512],
                                lhsT=b10m1x2[0:n_bl, 0:n_gxy],
                                rhs=blt[0:n_bl, 1 + lo:1 + lo + 512],
                                start=False, stop=False)
                            nc.tensor.matmul(
                                out=gy[0:n_gxy, lo:lo + 512],
                                lhsT=b10m1[0:n_bl, 0:n_gxy],
                                rhs=blt[0:n_bl, lo:lo + 512],
                                start=False, stop=sp)
                    nc.scalar.square(out=sqx_st[0:n_gxy, c * W:(c + 1) * W],
                                     in_=gx[0:n_gxy, :])
                    nc.scalar.square(out=sqy_st[0:n_gxy, c * W:(c + 1) * W],
                                     in_=gy[0:n_gxy, :])
                    # gxs/gys accumulation in f32 SBUF (masks need f32)
                    if c == 0:
                        nc.scalar.copy(out=gxs_sb[0:n_gxy, :],
                                       in_=gx[0:n_gxy, :])
                        nc.scalar.copy(out=gys_sb[0:n_gxy, :],
                                       in_=gy[0:n_gxy, :])
                    else:
                        nc.vector.tensor_tensor(out=gxs_sb[0:n_gxy, :],
                                                in0=gxs_sb[0:n_gxy, :],
                                                in1=gx[0:n_gxy, :],
                                                op=Alu.add)
                        nc.vector.tensor_tensor(out=gys_sb[0:n_gxy, :],
                                                in0=gys_sb[0:n_gxy, :],
                                                in1=gy[0:n_gxy, :],
                                                op=Alu.add)

                def finish_image(n_gxy, sqx_st, sqy_st, gm_t):
                    """magnitude: m2 (Pool), sqrt (Act), gm chunk adds."""
                    nc.gpsimd.tensor_tensor(out=sqx_st[0:n_gxy, :],
                                            in0=sqx_st[0:n_gxy, :],
                                            in1=sqy_st[0:n_gxy, :],
                                            op=Alu.add)
                    # reuse sqy's buffer: m2 (its last reader) just finished
                    mag = sqpool.tile([122, 3 * W], f32, tag="sqy",
                                      name="mag")
                    nc.scalar.sqrt(out=mag[0:n_gxy, :], in_=sqx_st[0:n_gxy, :])
                    gmi = gm_t[0:n_gxy, 1:W + 1]
                    nc.vector.tensor_tensor(out=gmi, in0=mag[0:n_gxy, 0:W],
                                            in1=mag[0:n_gxy, W:2 * W],
                                            op=Alu.add)
                    nc.vector.tensor_tensor(out=gmi, in0=gmi,
                                            in1=mag[0:n_gxy, 2 * W:3 * W],
                                            op=Alu.add)

                def make_masks(gxs_sb, gys_sb, n, shift, n_thin, j):
                    """u8 class masks at conv frame [0:n], DMA-shifted down
                    by `shift` rows into persistent thin-frame tiles."""
                    a2 = gspool.tile([122, W], f32, tag="a2", name="a2",
                                     bufs=1)
                    b2 = gspool.tile([122, W], f32, tag="b2", name="b2",
                                     bufs=1)
                    nc.scalar.square(out=a2[0:n, :], in_=gxs_sb[0:n, :])
                    nc.scalar.square(out=b2[0:n, :], in_=gys_sb[0:n, :])
                    sgx = gspool.tile([122, W], u8, tag="sgx", name="sgx",
                                      bufs=1)
                    sgy = gspool.tile([122, W], u8, tag="sgy", name="sgy",
                                      bufs=1)
                    nc.vector.tensor_scalar(out=sgx[0:n, :],
                                            in0=gxs_sb[0:n, :], scalar1=0.0,
                                            scalar2=None, op0=Alu.is_ge)
                    nc.vector.tensor_scalar(out=sgy[0:n, :],
                                            in0=gys_sb[0:n, :], scalar1=0.0,
                                            scalar2=None, op0=Alu.is_ge)
                    tmp = [msktpool.tile([122, W], u8, tag=t, name=t)
                           for t in ("tc0", "tc2", "tsm")]
                    nc.vector.scalar_tensor_tensor(
                        out=tmp[0][0:n, :], in0=a2[0:n, :], scalar=T22SQ,
                        in1=b2[0:n, :], op0=Alu.mult, op1=Alu.is_gt)
                    nc.vector.scalar_tensor_tensor(
                        out=tmp[1][0:n, :], in0=b2[0:n, :], scalar=T22SQ,
                        in1=a2[0:n, :], op0=Alu.mult, op1=Alu.is_gt)
                    # sign agreement == (ab >= 0) wherever c0/c2 don't apply
                    nc.vector.tensor_tensor(out=tmp[2][0:n, :],
                                            in0=sgx[0:n, :], in1=sgy[0:n, :],
                                            op=Alu.is_equal)
                    out = []
                    for t, tag in zip(tmp, ("c0", "c2", "sm")):
                        p = mskpool.tile([n_thin, W], u8, tag=f"{tag}_{j}",
                                         name=f"{tag}_{j}")
                        nc.sync.dma_start(out=p[:],
                                          in_=t[shift:shift + n_thin, :])
                        out.append(p)
                    return out

                # ---- B8 block FIRST so the collectives hide under main conv
                nc.vector.memset(gm8[:], 0.0)
                sqx8 = sqpool.tile([122, 3 * W], f32, tag="sqx", name="sqx")
                sqy8 = sqpool.tile([122, 3 * W], f32, tag="sqy", name="sqy")
                gxs8 = gspool.tile([122, W], f32, tag="gxs", name="gxs")
                gys8 = gspool.tile([122, W], f32, tag="gys", name="gys")
                for c in range(3):
                    conv_pipeline(c, 88, 86, 84, mt["BV_8"], mt["BVG1_8"],
                                  mt["BVG2_8"], mt["B121_8"], mt["B121N_8"],
                                  mt["B10M1_8"], mt["B10M1X2_8"],
                                  x8h[c], x8l[c], sqx8, sqy8, gxs8, gys8)
                finish_image(84, sqx8, sqy8, gm8)
                # thin8 frame = conv rows 3..83 -> shift 3, 81 rows
                m8 = make_masks(gxs8, gys8, 84, 3, 81, 8)

                ag_in = dpool.tile([81, W], f32, tag="ag_in")
                ag_out = dpool.tile([B * 81, W], f32, tag="ag_out")
                nc.sync.dma_start(out=ag_in[:], in_=gm8[3:84, 1:W + 1])
                nc.gpsimd.collective_compute(
                    "AllGather", Alu.bypass,
                    replica_groups=[list(range(NC))],
                    ins=[ag_in.opt()], outs=[ag_out.opt()])
                a2a_in = dpool.tile([B * 81, W], f32, tag="a2a_in")
                a2a_out = dpool.tile([B * 81, W], f32, tag="a2a_out")
                for b in range(B):
                    dr, dc = DELTAS[b]
                    nc.sync.dma_start(
                        out=a2a_in[81 * b:81 * (b + 1), :],
                        in_=gm8[3 + dr:84 + dr, 1 + dc:W + 1 + dc])
                nc.gpsimd.collective_compute(
                    "AllToAll", Alu.bypass,
                    replica_groups=[list(range(NC))],
                    ins=[a2a_in.opt()], outs=[a2a_out.opt()])
                nc.sync.dma_start(out=ce8[:], in_=gm8[3:84, 1:W + 1])

                # ---- main slab: 8 images x 3 channels
                earlyP = {}

                def emit_early(k, bs):
                    """P[k] for directions bs, computed during the conv tail
                    (images k and k+4 are already done). Both compares on
                    DVE (conv-idle); the and on Pool."""
                    for b in bs:
                        dr, dc = DELTAS[b]
                        nt = 121 if b <= 4 else 120
                        Cs = []
                        for j in (k, k + 4):
                            if b <= 4:
                                i0 = gm_tiles[j][0:nt, 1:W + 1]
                                i1 = (gm_tiles[j][0:nt, 1 + dc:W + 1 + dc]
                                      if dr == 0 else
                                      ce[j][0:nt, 1 + dc:W + 1 + dc])
                            else:
                                i0 = ce[j][0:nt, 1:W + 1]
                                i1 = gm_tiles[j][0:nt, 1 + dc:W + 1 + dc]
                            cj = gmpool.tile([121, W], f16, tag="cE",
                                             name="cE", bufs=2)
                            nc.vector.tensor_tensor(out=cj[0:nt, :], in0=i0,
                                                    in1=i1, op=Alu.is_gt)
                            Cs.append(cj)
                        pk = gmpool.tile([121, W], f16, tag=f"pE{k}_{b}",
                                         name=f"pE{k}_{b}")
                        nc.gpsimd.tensor_tensor(out=pk[0:nt, :],
                                                in0=Cs[0][0:nt, :],
                                                in1=Cs[1][0:nt, :],
                                                op=Alu.mult)
                        earlyP[(b, k)] = pk

                for j in range(B):
                    gm_j = gmpool.tile([122, W + 2], f32, tag=f"gm{j}",
                                       name=f"gm{j}")
                    nc.gpsimd.memset(gm_j[:, 0:1], 0.0)
                    nc.gpsimd.memset(gm_j[:, W + 1:W + 2], 0.0)
                    sqx_st = sqpool.tile([122, 3 * W], f32, tag="sqx",
                                         name="sqx")
                    sqy_st = sqpool.tile([122, 3 * W], f32, tag="sqy",
                                         name="sqy")
                    gxs_sb = gspool.tile([122, W], f32, tag="gxs",
                                         name="gxs")
                    gys_sb = gspool.tile([122, W], f32, tag="gys",
                                         name="gys")
                    for c in range(3):
                        conv_pipeline(c, 128, 124, 122, mt["BV"], mt["BVG1"],
                                      mt["BVG2"], mt["B121"], mt["B121N"],
                                      mt["B10M1"], mt["B10M1X2"],
                                      xh[3 * j + c], xl[3 * j + c],
                                      sqx_st, sqy_st, gxs_sb, gys_sb)
                    finish_image(122, sqx_st, sqy_st, gm_j)
                    gm_tiles.append(gm_j)
                    # dirs 0-4 (shifted frame): masks shift 0, 121 rows;
                    # dirs 5-7 (thin frame): masks shift 1, 120 rows.
                    if j <= 4:
                        masks.append(make_masks(gxs_sb, gys_sb, 122, 0,
                                                121, j))
                    else:
                        masks.append(make_masks(gxs_sb, gys_sb, 122, 1,
                                                120, j))
                    cet = gmpool.tile([121, W + 2], f32, tag=f"ce{j}",
                                      name=f"ce{j}")
                    nc.sync.dma_start(out=cet[:], in_=gm_j[1:122, :])
                    ce.append(cet)
                    # k=0 pair (images 0,4) ready after image 4; split the
                    # emission across two images to avoid head-of-line
                    # stalls of the next image's conv DVE ops.
                    if j == 4:
                        emit_early(0, (0, 1, 2))
                    elif j == 5:
                        emit_early(0, (3, 4, 5))
                    elif j == 6:
                        emit_early(0, (6, 7))
                        emit_early(1, (0,))
                    elif j == 7:
                        emit_early(1, (1, 2, 3, 4, 5, 6, 7))

            # =========== NMS phase ========================================
            # thin frame (dirs 5-7): row p <-> img 118i-1+p, 120 rows.
            # shifted frame (dirs 0-4): row r <-> thin p=r-1, 121 rows.
            with (
                tc.tile_pool(name="cmap", bufs=4) as cpool2,
                tc.tile_pool(name="pmap", bufs=1) as ppool,
                tc.tile_pool(name="g8p", bufs=3) as g8pool,
                tc.tile_pool(name="nmst", bufs=2) as npool,
                tc.tile_pool(name="outp", bufs=2) as opool,
                tc.tile_pool(name="psC", bufs=2, space="PSUM") as psC,
            ):
                def nms_stage1(b_masks, gm_b, get_in0, get_in1, n_thin,
                               bones, n_mp, out_lo, out_dram, n_out,
                               pre=None):
                    """Compares + P-ands (DVE+Pool). Returns state for
                    stage2; two-stage emission keeps the in-order DVE queue
                    fed while Pool computes the ands."""
                    P = []
                    for k in range(4):
                        if pre is not None and k in pre:
                            P.append(pre[k])
                            continue
                        Cs = []
                        for j in (k, k + 4):
                            cj = cpool2.tile([121, W], f16, tag="c", name="c")
                            if k % 2 == 0 and j == k + 4:
                                # route through idle Pool: d = a-b (Pool),
                                # C = d > 0 (DVE ts, 2x mode)
                                d = cpool2.tile([121, W], f32, tag="d",
                                                name="d", bufs=2)
                                nc.gpsimd.tensor_tensor(
                                    out=d[0:n_thin, :], in0=get_in0(j),
                                    in1=get_in1(j), op=Alu.subtract)
                                nc.vector.tensor_scalar(
                                    out=cj[0:n_thin, :], in0=d[0:n_thin, :],
                                    scalar1=0.0, scalar2=None, op0=Alu.is_gt)
                            else:
                                nc.vector.tensor_tensor(
                                    out=cj[0:n_thin, :], in0=get_in0(j),
                                    in1=get_in1(j), op=Alu.is_gt)
                            Cs.append(cj)
                        tag = "psel" if k == 3 else f"p{k}"
                        pk = ppool.tile([121, W], f16, tag=tag, name=tag,
                                        bufs=2)
                        # and of {0,1} masks == product (Pool has no
                        # logical ops)
                        nc.gpsimd.tensor_tensor(
                            out=pk[0:n_thin, :], in0=Cs[0][0:n_thin, :],
                            in1=Cs[1][0:n_thin, :], op=Alu.mult)
                        P.append(pk)
                    return (P, b_masks, gm_b, n_thin, bones, n_mp, out_lo,
                            out_dram, n_out)

                def nms_stage2(st):
                    """Select by class masks + thresholds + hysteresis."""
                    (P, b_masks, gm_b, n_thin, bones, n_mp, out_lo,
                     out_dram, n_out) = st
                    c0, c2, sm = b_masks
                    psel = P[3]
                    nc.vector.copy_predicated(out=psel[0:n_thin, :],
                                              mask=sm[0:n_thin, :],
                                              data=P[1][0:n_thin, :])
                    nc.vector.copy_predicated(out=psel[0:n_thin, :],
                                              mask=c0[0:n_thin, :],
                                              data=P[0][0:n_thin, :])
                    nc.vector.copy_predicated(out=psel[0:n_thin, :],
                                              mask=c2[0:n_thin, :],
                                              data=P[2][0:n_thin, :])
                    tq = npool.tile([121, W], f16, tag="tq", name="tq")
                    th = npool.tile([121, W], f16, tag="th", name="th")
                    nc.vector.tensor_scalar(out=tq[0:n_thin, :], in0=gm_b,
                                            scalar1=LOW_S, scalar2=None,
                                            op0=Alu.is_ge)
                    nc.vector.tensor_scalar(out=th[0:n_thin, :], in0=gm_b,
                                            scalar1=HIGH_S, scalar2=None,
                                            op0=Alu.is_gt)
                    q = npool.tile([121, W], f16, tag="q", name="q")
                    nc.vector.tensor_tensor(out=q[0:n_thin, :],
                                            in0=tq[0:n_thin, :],
                                            in1=psel[0:n_thin, :],
                                            op=Alu.logical_and)
                    strong = npool.tile([121, W + 2], f16, tag="strong",
                                        name="strong")
                    nc.gpsimd.memset(strong[:, 0:1], 0.0)
                    nc.gpsimd.memset(strong[:, W + 1:W + 2], 0.0)
                    nc.vector.tensor_tensor(out=strong[0:n_thin, 1:W + 1],
                                            in0=th[0:n_thin, :],
                                            in1=q[0:n_thin, :],
                                            op=Alu.logical_and)
                    # mp = 3x3 box sum of strong: 3 col-shifted matmuls
                    mp = psC.tile([120, W], f32, tag="mp", name="mp")
                    for lo2 in (0, 512):
                        for t in range(3):
                            nc.tensor.matmul(
                                out=mp[0:n_mp, lo2:lo2 + 512],
                                lhsT=bones[0:n_thin, 0:n_mp],
                                rhs=strong[0:n_thin, t + lo2:t + lo2 + 512],
                                start=(t == 0), stop=(t == 2))
                    ot = opool.tile([120, W], f16, tag="ot", name="ot")
                    nc.vector.scalar_tensor_tensor(
                        out=ot[0:n_mp, :], in0=mp[0:n_mp, :], scalar=0.5,
                        in1=q[0:n_mp, :], op0=Alu.is_ge, op1=Alu.logical_and)
                    nc.gpsimd.memset(ot[0:n_mp, 0:1], 0.0)
                    nc.gpsimd.memset(ot[0:n_mp, W - 1:W], 0.0)
                    nc.sync.dma_start(out=out_dram,
                                      in_=ot[out_lo:out_lo + n_out, :])

                def nms_b(b):
                    dr, dc = DELTAS[b]
                    if b <= 4:
                        # shifted frame, 121 rows: in0 = raw gm rows 0..120,
                        # +1-row shift = ce (gm rows 1..121)
                        def in0(j):
                            return gm_tiles[j][0:121, 1:W + 1]

                        def in1(j):
                            if dr == 0:
                                return gm_tiles[j][0:121, 1 + dc:W + 1 + dc]
                            return ce[j][0:121, 1 + dc:W + 1 + dc]

                        return nms_stage1(masks[b],
                                          gm_tiles[b][0:121, 1:W + 1],
                                          in0, in1, 121, mt["BONES_D"], 120,
                                          2, outm[b], SLAB,
                                          pre={k: earlyP[(b, k)]
                                               for k in range(4)
                                               if (b, k) in earlyP})
                    else:
                        # thin frame, 120 rows: in0 = ce rows 0..119
                        # (gm rows 1..120), -1-row shift = raw gm rows 0..119
                        def in0(j):
                            return ce[j][0:120, 1:W + 1]

                        def in1(j):
                            return gm_tiles[j][0:120, 1 + dc:W + 1 + dc]

                        return nms_stage1(masks[b], ce[b][0:120, 1:W + 1],
                                          in0, in1, 120, mt["BONES"], 119,
                                          1, outm[b], SLAB,
                                          pre={k: earlyP[(b, k)]
                                               for k in range(4)
                                               if (b, k) in earlyP})

                # B8: own image only; shifted operands came via AllToAll.
                # thin8 frame: row p (base 0) <-> img 943+p, 81 rows.
                def load8(dram_src, tag):
                    def get(j):
                        t = g8pool.tile([81, W], f32, tag=tag, name=tag,
                                        bufs=2)
                        nc.sync.dma_start(
                            out=t[:], in_=dram_src[81 * j:81 * (j + 1), :])
                        return t[:]
                    return get

                prev = None
                for b in (0, 4, 1, 2, 3, 5, 6, 7, 8):  # 8 = B8 band
                    if b == 8:
                        st = nms_stage1(m8, ce8[:], load8(ag_out, "g8p"),
                                        load8(a2a_out, "g8s"), 81,
                                        mt["BONES8"], 81, 1, out8[:],
                                        B8_ROWS)
                    else:
                        st = nms_b(b)
                    if prev is not None:
                        nms_stage2(prev)
                    prev = st
                nms_stage2(prev)

    _legalize_waits(nc)
    _CACHE["nc"] = nc
    return nc


def _legalize_waits(nc):
    """Several ISA encodings (S2S2D2_STT, HWDGE DMACopy, ...) hold only one
    embedded sync-wait, but Tile's scheduler can attach more. Hoist all
    embedded waits of multi-wait instructions into a NoOp injected just
    before them on the same engine queue (NoOps carry many waits fine)."""
    import concourse.mybir as mybir
    n = 0
    for f in nc.m.functions:
        for blk in f.blocks:
            out = []
            for ins in blk.instructions:
                si = ins.sync_info
                if (si is not None and si.on_wait is not None
                        and len(si.on_wait) > 1):
                    for w in si.on_wait:
                        nop = mybir.InstNoOp(
                            name=f"WFIX-{n}", engine=ins.engine,
                            sync_info=mybir.SyncInfo(on_wait=[w],
                                                     on_update=[]))
                        n += 1
                        out.append(nop)
                    ins.sync_info = mybir.SyncInfo(
                        on_wait=[],
                        on_update=list(si.on_update or []))
                out.append(ins)
            blk.instructions = out


def _in_maps(img):
    img = np.asarray(img, dtype=np.float32)
    hi = img.astype(np.float16)
    lo = (img - hi.astype(np.float32)).astype(np.float16)
    pad = np.zeros((B, 3, 5, W), np.float16)
    hip = np.concatenate([pad, hi], axis=2)  # rows shifted by +5
    lop = np.concatenate([pad, lo], axis=2)
    maps = []
    for i in range(NC):
        r0 = SLAB * i  # padded row index of img row 118i-5
        m = {"xh": np.ascontiguousarray(
                 hip[:, :, r0:r0 + 128, :].reshape(B * 3, 128, W)),
             "xl": np.ascontiguousarray(
                 lop[:, :, r0:r0 + 128, :].reshape(B * 3, 128, W)),
             "x8h": np.ascontiguousarray(hi[i, :, B8_START - 8:, :]),
             "x8l": np.ascontiguousarray(lo[i, :, B8_START - 8:, :])}
        m.update(_const_mats(i))
        maps.append(m)
    return maps


def kernel(img, gauss_h=None, gauss_v=None, sobel_h=None, sobel_v=None,
           dir_f=None, connect_f=None, _want_time=False):
    from concourse.bass_utils import run_bass_kernel_spmd
    nc = _build_program()
    maps = _in_maps(np.asarray(img))
    res = run_bass_kernel_spmd(nc, maps, list(range(NC)), trace=_want_time)
    out = np.zeros((B, 1, H, W), np.float32)
    for i in range(NC):
        r = res.results[i]
        out[:, 0, SLAB * i:SLAB * (i + 1), :] = \
            np.asarray(r["outm"], np.float32)
        out[i, 0, B8_START:, :] = np.asarray(r["out8"], np.float32)
    if _want_time:
        return out, res
    return out
